# revision 1
# baseline (speedup 1.0000x reference)
"""AFNO transformer block on 8 Trainium2 NeuronCores (bf16).

Distribution:
  Phase 1 (channel-block sharded): core k owns channels [96k, 96k+96).
    z loaded once in bf16 as [90w, 96c, 90h] tiles per (b, wc-half); LN1
    partial stats (reduce over c) -> per-batch AllReduce (token-major
    [2, 16200]) -> LN1 applied in place -> spectral path: F1 (W-DFT,
    flip), F2 (H-DFT, flip, r/i packed into one PSUM), block complex MLP
    (layer1 weight-stationary, layer2 flip packed), inverse H-DFT (flip,
    packed), inverse W-DFT (weight-stationary over wf).
  Two AllToAlls (one per batch, bf16), overlapped: a2a_0 runs during
    b=1's spectral chain, a2a_1 during phase-2 b=0 tiles.
  Phase 2 (token sharded): core j owns tokens [2025j, 2025(j+1)) of each
    batch. h = filt + LN1(x) + x assembled in bf16, LN2 folded into fc1
    (uneg rank-1 matmul + r2 broadcast), fc1 -> Gelu -> fc2 -> residual
    -> strided DMA straight to token-major output.
"""
import math
import numpy as np
import ml_dtypes

import concourse.bass as bass
import concourse.mybir as mybir
import concourse.tile as tile
from concourse import bacc
from concourse.bass_utils import run_bass_kernel_spmd

F32 = mybir.dt.float32
BF16 = mybir.dt.bfloat16
AF = mybir.ActivationFunctionType
OP = mybir.AluOpType
AX = mybir.AxisListType

NCORES = 8
B, H, W, C = 2, 90, 180, 768
BS = 96            # channels per core / AFNO block size
KW = 46            # kept W-frequency modes
HID = 3072
LAM = 0.01
EPS = 1e-5
TOKB = H * W       # 16200 tokens per batch
TSB = TOKB // NCORES   # 2025 tokens per (core, batch)
TSH = 2 * TSB      # 4050 tokens per core
NM = KW * H        # 4140 modes per block
SQN = math.sqrt(H * W)
NCC = 6            # 768/128
NMO = 24           # 3072/128
TT = 405           # phase-2 token tile width
NT = TSB // TT     # 5 tiles per batch
M1CH = 460         # MLP1 chunk (4140 = 9*460)
BF = ml_dtypes.bfloat16


def _dft_consts():
    wv = np.arange(W, dtype=np.float64)[:, None]
    wf = np.arange(KW, dtype=np.float64)[None, :]
    ang = 2.0 * np.pi * wv * wf / W
    fwr = np.cos(ang) / math.sqrt(W)
    fwi = -np.sin(ang) / math.sqrt(W)
    fwpack = np.concatenate([fwr, fwi], axis=1)          # (180, 92)
    hv = np.arange(H, dtype=np.float64)[:, None]
    hf = np.arange(H, dtype=np.float64)[None, :]
    angh = 2.0 * np.pi * hv * hf / H
    fhc = np.cos(angh) / math.sqrt(H)
    fhs = np.sin(angh) / math.sqrt(H)
    fhsm = -fhs
    alpha = np.ones(KW); alpha[1:] = 2.0
    iwr = alpha[None, :] * np.cos(ang) / math.sqrt(W)    # (180, 46)
    iwi = -alpha[None, :] * np.sin(ang) / math.sqrt(W)
    iwrt = np.ascontiguousarray(iwr.T)                   # (46, 180)
    iwit = np.ascontiguousarray(iwi.T)
    c = {}
    c["fwp0"] = fwpack[:90]
    c["fwp1"] = fwpack[90:]
    c["f2a"] = np.concatenate([fhc, fhsm], axis=1)       # (90, 180)
    c["f2b"] = np.concatenate([fhs, fhc], axis=1)
    c["iha"] = np.concatenate([fhc, fhs], axis=1)
    c["ihb"] = np.concatenate([fhsm, fhc], axis=1)
    c["iwrt"] = iwrt
    c["iwit"] = iwit
    return {k: np.ascontiguousarray(v).astype(BF) for k, v in c.items()}


def _send_pieces(j):
    """(h0,h1,w0,w1) global-w pieces covering dest j's tokens of a batch."""
    s0, e0 = TSB * j, TSB * (j + 1)
    pieces, t = [], s0
    while t < e0:
        h = t // W
        w0 = t - h * W
        if w0 != 0 or e0 - t < W:
            w1 = min(W, w0 + (e0 - t))
            pieces.append((h, h + 1, w0, w1))
            t += w1 - w0
        else:
            h1 = min(H, h + (e0 - t) // W)
            pieces.append((h, h1, 0, W))
            t += (h1 - h) * W
    return pieces


def _recv_pieces(cc):
    c0, out, r0 = cc * 128, [], 0
    while r0 < 128:
        s = (c0 + r0) // BS
        ci = (c0 + r0) % BS
        n = min(BS - ci, 128 - r0)
        out.append((r0, s, ci, n))
        r0 += n
    return out


_CACHE = {}


def _build_nc():
    if "nc" in _CACHE:
        return _CACHE["nc"]
    nc = bacc.Bacc("TRN2", target_bir_lowering=False, debug=False,
                   num_devices=NCORES)

    def g(n, s, dt=BF16):
        return nc.dram_tensor(n, s, dt, kind="ExternalInput")

    xw = g("xw", [W, B, H, BS])
    xc = g("xc", [C, TSH])
    fwp0 = g("fwp0", [90, 92]); fwp1 = g("fwp1", [90, 92])
    f2a = g("f2a", [90, 180]); f2b = g("f2b", [90, 180])
    iha = g("iha", [90, 180]); ihb = g("ihb", [90, 180])
    iwrt = g("iwrt", [KW, W]); iwit = g("iwit", [KW, W])
    w1r = g("w1r", [BS, BS]); w1i = g("w1i", [BS, BS]); w1im = g("w1im", [BS, BS])
    b1r = g("b1r", [BS, 1], F32); b1i = g("b1i", [BS, 1], F32)
    b1sr = g("b1sr", [BS, 1], F32); b1si = g("b1si", [BS, 1], F32)
    w2p1 = g("w2p1", [BS, 192]); w2p2 = g("w2p2", [BS, 192])
    b2pk = g("b2pk", [1, 192])
    fc1m = g("fc1m", [C, HID])
    uneg = g("uneg", [1, HID])
    gbias = g("gbias", [128, NMO], F32)
    fc2w = g("fc2w", [HID, C])
    fc2b = g("fc2b", [128, NCC], F32)
    g1f = g("g1f", [128, NCC], F32); be1f = g("be1f", [128, NCC], F32)
    ones1 = g("ones1", [1, 128])
    ones128 = g("ones128", [128, 1])

    out = nc.dram_tensor("out", [C, TSH], F32, kind="ExternalOutput")
    rg = [list(range(NCORES))]

    from contextlib import ExitStack
    with tile.TileContext(nc) as tc:
        with ExitStack() as st0:
            cp = st0.enter_context(tc.tile_pool(name="const", bufs=1))
            dram = st0.enter_context(tc.tile_pool(name="dram", bufs=1, space="DRAM"))

            def cl(t, shape, dt=BF16):
                nm = f"c_{t.name}"
                s = cp.tile(shape, dt, name=nm, tag=nm)
                nc.sync.dma_start(s[:], t[:])
                return s

            c_fwp0 = cl(fwp0, [90, 92]); c_fwp1 = cl(fwp1, [90, 92])
            c_f2a = cl(f2a, [90, 180]); c_f2b = cl(f2b, [90, 180])
            c_iha = cl(iha, [90, 180]); c_ihb = cl(ihb, [90, 180])
            c_iwrt = cl(iwrt, [KW, W]); c_iwit = cl(iwit, [KW, W])
            c_w1r = cl(w1r, [BS, BS]); c_w1i = cl(w1i, [BS, BS])
            c_w1im = cl(w1im, [BS, BS])
            c_b1r = cl(b1r, [BS, 1], F32); c_b1i = cl(b1i, [BS, 1], F32)
            c_b1sr = cl(b1sr, [BS, 1], F32); c_b1si = cl(b1si, [BS, 1], F32)
            c_w2p1 = cl(w2p1, [BS, 192]); c_w2p2 = cl(w2p2, [BS, 192])
            c_b2pk = cl(b2pk, [1, 192])
            c_gbias = cl(gbias, [128, NMO], F32)
            c_fc2b = cl(fc2b, [128, NCC], F32)
            c_g1f = cl(g1f, [128, NCC], F32); c_be1f = cl(be1f, [128, NCC], F32)
            c_uneg = cl(uneg, [1, HID])
            c_ones1 = cl(ones1, [1, 128]); c_ones128 = cl(ones128, [128, 1])
            c_eps = cp.tile([128, 1], F32, name="c_eps")
            nc.vector.memset(c_eps[:], EPS)

            st_in = [dram.tile([2, TOKB], F32, name=f"st_in{b_}") for b_ in range(B)]
            st_out = [dram.tile([2, TOKB], F32, name=f"st_out{b_}") for b_ in range(B)]
            a2a_in = [dram.tile([NCORES, BS, TSB], BF16, name=f"a2a_in{b_}")
                      for b_ in range(B)]
            a2a_out = [dram.tile([NCORES, BS, TSB], BF16, name=f"a2a_out{b_}")
                       for b_ in range(B)]
            rows_dram = dram.tile([B, 2, TSB], F32, name="rows_dram")

            # ================= phase 1 =================
            with ExitStack() as st1:
                sqp = st1.enter_context(tc.tile_pool(name="sqp", bufs=2))
                stp = st1.enter_context(tc.tile_pool(name="stats", bufs=1))
                zp = st1.enter_context(tc.tile_pool(name="zp", bufs=2))
                clp = st1.enter_context(tc.tile_pool(name="clp", bufs=2))
                zbp = st1.enter_context(tc.tile_pool(name="zbp", bufs=1))
                ybo2 = st1.enter_context(tc.tile_pool(name="ybo2", bufs=2))
                o1p = st1.enter_context(tc.tile_pool(name="o1p", bufs=1))
                u2p = st1.enter_context(tc.tile_pool(name="u2p", bufs=1))
                s2p = st1.enter_context(tc.tile_pool(name="s2p", bufs=1))
                pp = st1.enter_context(tc.tile_pool(name="psum1", bufs=8,
                                                    space="PSUM"))
                zhs = {}

                def stk(t, kind):
                    return bass.AP(tensor=t[:].tensor,
                                   offset=t[:].offset + kind * TOKB,
                                   ap=[[90, 90], [8100, 2], [1, 90]])

                def emit_loads_stats(b):
                    """Load z (bf16), partial LN1 stats, AllReduce trigger."""
                    eng = nc.vector
                    zh = []
                    for wc in range(2):
                        zt = zp.tile([90, H, BS], BF16, tag="zh",
                                     name=f"zh{b}{wc}")
                        eng_ld = nc.scalar if b == 0 else nc.sync
                        eng_ld.dma_start(
                            zt[:], xw[wc * 90:(wc + 1) * 90, b, :, :])
                        zh.append(zt)
                    zhs[b] = zh
                    s_sum = stp.tile([90, 2, H], F32, tag="ssum")
                    s_sq = stp.tile([90, 2, H], F32, tag="ssq")
                    s_t = stp.tile([90, H], F32, tag="st_t")
                    zhs[b, "sum"] = s_sum
                    zhs[b, "sq"] = s_sq
                    for wc in range(2):
                        zt = zh[wc]
                        eng.reduce_sum(s_sum[:, wc, :], zt[:], axis=AX.X)
                        # squared sums in 24-channel blocks (small scratch)
                        for blk in range(4):
                            sqt = sqp.tile([90, H, 24], BF16, tag="sqt")
                            zsl = zt[:, :, blk * 24:(blk + 1) * 24]
                            if b == 0:
                                nc.scalar.activation(out=sqt[:], in_=zsl,
                                                     func=AF.Square)
                            else:
                                nc.gpsimd.tensor_mul(sqt[:], zsl, zsl)
                            if blk == 0:
                                eng.reduce_sum(s_sq[:, wc, :], sqt[:], axis=AX.X)
                            else:
                                eng.reduce_sum(s_t[:], sqt[:], axis=AX.X)
                                eng.tensor_add(s_sq[:, wc, :], s_sq[:, wc, :],
                                               s_t[:])
                    nc.sync.dma_start(stk(st_in[b], 0), s_sum[:])
                    nc.sync.dma_start(stk(st_in[b], 1), s_sq[:])
                    nc.gpsimd.collective_compute(
                        "AllReduce", OP.add, replica_groups=rg,
                        ins=[st_in[b][:].opt()], outs=[st_out[b][:].opt()])

                def emit_post_stats(b):
                    """st recv, m/r, phase-2 rows, LN1 apply in place."""
                    s_sum, s_sq = zhs[b, "sum"], zhs[b, "sq"]
                    nc.sync.dma_start(s_sum[:], stk(st_out[b], 0))
                    nc.sync.dma_start(s_sq[:], stk(st_out[b], 1))
                    s_m = stp.tile([90, 2, H], F32, tag="sm")
                    s_r = stp.tile([90, 2, H], F32, tag="sr")
                    s_v = stp.tile([90, 2, H], F32, tag="sv")
                    nc.vector.tensor_scalar(out=s_m[:], in0=s_sum[:],
                                            scalar1=1.0 / C, scalar2=None,
                                            op0=OP.mult)
                    nc.vector.tensor_scalar(out=s_r[:], in0=s_sq[:],
                                            scalar1=1.0 / C, scalar2=None,
                                            op0=OP.mult)
                    nc.vector.tensor_mul(s_v[:], s_m[:], s_m[:])
                    nc.vector.tensor_sub(s_r[:], s_r[:], s_v[:])
                    nc.scalar.activation(out=s_r[:], in_=s_r[:],
                                         func=AF.Sqrt, bias=c_eps[:90])
                    nc.vector.reciprocal(s_r[:], s_r[:])
                    s_rb = stp.tile([90, 2, H], BF16, tag="srb")
                    s_mrb = stp.tile([90, 2, H], BF16, tag="smrb")
                    nc.vector.tensor_copy(s_rb[:], s_r[:])
                    nc.vector.tensor_mul(s_v[:], s_m[:], s_r[:])
                    nc.vector.tensor_copy(s_mrb[:], s_v[:])

                    def bc(t, wc, n):
                        a = t[:, wc, :]
                        return bass.AP(tensor=a.tensor, offset=a.offset,
                                       ap=[list(a.ap[0]), [1, H], [0, n]])
                    eng_ln = nc.gpsimd if b == 0 else nc.vector
                    for c0 in range(0, BS, 8):
                        for wc in range(2):
                            zv = zhs[b][wc][:, :, c0:c0 + 8]
                            eng_ln.tensor_mul(zv, zv, bc(s_rb, wc, 8))
                    # F1 of the (channel-independent) m*r field, for the
                    # mean-subtraction folded into the F1 copies
                    fmr = stp.tile([90, 92], F32, tag="fmr", name="fmr")
                    pfm = pp.tile([90, 92], F32, tag="pp", name="psfmr")
                    nc.tensor.matmul(pfm[:], s_mrb[:, 0, :], c_fwp0[:],
                                     start=True, stop=False)
                    nc.tensor.matmul(pfm[:], s_mrb[:, 1, :], c_fwp1[:],
                                     start=False, stop=True)
                    nc.vector.tensor_copy(fmr[:], pfm[:])
                    zhs[b, "fmr"] = fmr

                def emit_f1(b):
                    zh = zhs[b]
                    fmr = zhs[b, "fmr"]
                    yb = ybo2.tile([90, BS, 92], BF16, tag="ybo2", name=f"yb{b}")
                    zhs[b, "yb"] = yb
                    for gi, c0 in enumerate(range(0, BS, 4)):
                        pf = pp.tile([90, 4 * 92], F32, tag="pp", name="psf1")
                        for ci in range(4):
                            c = c0 + ci
                            nc.tensor.matmul(pf[:, ci * 92:(ci + 1) * 92],
                                             zh[0][:, :, c], c_fwp0[:],
                                             start=True, stop=False)
                            nc.tensor.matmul(pf[:, ci * 92:(ci + 1) * 92],
                                             zh[1][:, :, c], c_fwp1[:],
                                             start=False, stop=True)
                        dst = yb[:, c0:c0 + 4, :]
                        src = pf[:].rearrange("p (a b) -> p a b", a=4)
                        fa = fmr[:]
                        fbc = bass.AP(tensor=fa.tensor, offset=fa.offset,
                                      ap=[list(fa.ap[0]), [0, 4], [1, 92]])
                        nc.vector.tensor_tensor(out=dst, in0=src, in1=fbc,
                                                op=OP.subtract)

                def emit_f2(b):
                    yb = zhs[b, "yb"]
                    # ---- F2 (flip, packed r/i)
                    zb = zbp.tile([BS, 2, KW, H], BF16, tag="zb", name=f"zb{b}")
                    zhs[b, "zb"] = zb
                    for wf in range(KW):
                        pz = pp.tile([BS, 180], F32, tag="pp", name="psf2")
                        nc.tensor.matmul(pz[:], yb[:, :, wf], c_f2a[:],
                                         start=True, stop=False)
                        nc.tensor.matmul(pz[:], yb[:, :, 46 + wf], c_f2b[:],
                                         start=False, stop=True)
                        if wf % 2 == 0:
                            nc.scalar.activation(
                                out=zb[:, :, wf, :],
                                in_=pz[:].rearrange("p (a b) -> p a b", a=2),
                                func=AF.Copy)
                        else:
                            nc.vector.tensor_copy(
                                zb[:, :, wf, :],
                                pz[:].rearrange("p (a b) -> p a b", a=2))

                def emit_rest(b):
                    zb = zhs[b, "zb"]
                    # ---- block MLP layer 1 (weight-stationary) + Relu
                    o1 = o1p.tile([BS, 2, NM], BF16, tag="o1", name=f"o1{b}")
                    zr_f = zb[:, 0].rearrange("p a b -> p (a b)")
                    zi_f = zb[:, 1].rearrange("p a b -> p (a b)")
                    for ch in range(9):
                        n0 = ch * M1CH
                        zr_s = zr_f[:, n0:n0 + M1CH]
                        zi_s = zi_f[:, n0:n0 + M1CH]
                        por = pp.tile([BS, M1CH], F32, tag="pp", name="pso1r")
                        nc.tensor.matmul(por[:], c_w1r[:], zr_s,
                                         start=True, stop=False)
                        nc.tensor.matmul(por[:], c_w1im[:], zi_s,
                                         start=False, stop=True)
                        poi = pp.tile([BS, M1CH], F32, tag="pp", name="pso1i")
                        nc.tensor.matmul(poi[:], c_w1i[:], zr_s,
                                         start=True, stop=False)
                        nc.tensor.matmul(poi[:], c_w1r[:], zi_s,
                                         start=False, stop=True)
                        if ch == 0:
                            # be1 spike contribution on mode (0,0) only
                            nc.vector.tensor_scalar(out=por[:, 0:1],
                                                    in0=por[:, 0:1],
                                                    scalar1=c_b1sr[:],
                                                    scalar2=None, op0=OP.add)
                            nc.vector.tensor_scalar(out=poi[:, 0:1],
                                                    in0=poi[:, 0:1],
                                                    scalar1=c_b1si[:],
                                                    scalar2=None, op0=OP.add)
                        nc.scalar.activation(out=o1[:, 0, n0:n0 + M1CH],
                                             in_=por[:], func=AF.Relu,
                                             bias=c_b1r[:])
                        nc.scalar.activation(out=o1[:, 1, n0:n0 + M1CH],
                                             in_=poi[:], func=AF.Relu,
                                             bias=c_b1i[:])

                    # ---- block MLP layer 2 (flip, packed) + softshrink
                    o2 = ybo2.tile([H, 2, KW, BS], BF16, tag="ybo2",
                                   name=f"o2{b}")
                    for wf in range(KW):
                        lr = o1[:, 0, wf * H:(wf + 1) * H]
                        li = o1[:, 1, wf * H:(wf + 1) * H]
                        pm = pp.tile([H, 192], F32, tag="pp", name="pso2")
                        nc.tensor.matmul(pm[:], lr, c_w2p1[:],
                                         start=True, stop=False)
                        nc.tensor.matmul(pm[:], li, c_w2p2[:],
                                         start=False, stop=False)
                        nc.tensor.matmul(pm[:], c_ones1[:, 0:H], c_b2pk[:],
                                         start=False, stop=True)
                        clip = clp.tile([H, 192], F32, tag="clip")
                        nc.vector.tensor_scalar(out=clip[:], in0=pm[:],
                                                scalar1=-LAM, scalar2=LAM,
                                                op0=OP.max, op1=OP.min)
                        nc.vector.tensor_tensor(
                            out=o2[:, :, wf, :],
                            in0=pm[:].rearrange("p (a b) -> p a b", a=2),
                            in1=clip[:].rearrange("p (a b) -> p a b", a=2),
                            op=OP.subtract)

                    # ---- inverse H-DFT (flip, packed) -> u2 [46, 2, 96, 90]
                    u2 = u2p.tile([KW, 2, BS, H], BF16, tag="u2", name=f"u2{b}")
                    for c in range(BS):
                        lr = o2[:, 0, :, c]
                        li = o2[:, 1, :, c]
                        pu = pp.tile([KW, 180], F32, tag="pp", name="psu")
                        nc.tensor.matmul(pu[:], lr, c_iha[:],
                                         start=True, stop=False)
                        nc.tensor.matmul(pu[:], li, c_ihb[:],
                                         start=False, stop=True)
                        dst = u2[:, :, c, :]
                        src = pu[:].rearrange("p (a b) -> p a b", a=2)
                        if c % 2 == 0:
                            nc.scalar.activation(out=dst, in_=src, func=AF.Copy)
                        else:
                            nc.vector.tensor_copy(dst, src)

                    # ---- inverse W-DFT (flip) -> s2 [90h, 96c, 180w]
                    s2 = s2p.tile([H, BS, W], BF16, tag="s2", name=f"s2{b}")
                    for c in range(BS):
                        pf = pp.tile([H, W], F32, tag="pp", name="psw")
                        nc.tensor.matmul(pf[:], u2[:, 0, c, :], c_iwrt[:],
                                         start=True, stop=False)
                        nc.tensor.matmul(pf[:], u2[:, 1, c, :], c_iwit[:],
                                         start=False, stop=True)
                        if c % 2 == 0:
                            nc.scalar.activation(out=s2[:, c, :], in_=pf[:],
                                                 func=AF.Copy)
                        else:
                            nc.vector.tensor_copy(s2[:, c, :], pf[:])

                    # ---- a2a send pieces (SBUF -> DRAM, w-contiguous)
                    for j in range(NCORES):
                        t0 = TSB * j
                        for (h0, h1, w0, w1) in _send_pieces(j):
                            src = s2[h0:h1, :, w0:w1]
                            dst = bass.AP(
                                tensor=a2a_in[b][:].tensor,
                                offset=(a2a_in[b][:].offset
                                        + j * BS * TSB
                                        + (h0 * W + w0 - t0)),
                                ap=[[W, h1 - h0], [TSB, BS], [1, w1 - w0]])
                            nc.sync.dma_start(dst, src)

                # emission order chosen so collective triggers never block
                # earlier-needed work on the same engine queue
                upf = st1.enter_context(tc.tile_pool(name="upf", bufs=8))
                upr = st1.enter_context(tc.tile_pool(name="upr", bufs=2))

                def emit_p2rows(tb, tts):
                    """Raw phase-2 LN1 sums from xc (PE ones-matmuls, used as
                    fillers between phase-1 stages to keep the PE warm).
                    Row math happens in the pipelined phase-2 prep."""
                    for tt in tts:
                        tg = tb * TSB + tt * TT
                        xus = []
                        for cc in range(NCC):
                            xu = upf.tile([128, TT], BF16, tag="xu", name="xu")
                            nc.sync.dma_start(
                                xu[:], xc[cc * 128:(cc + 1) * 128, tg:tg + TT])
                            xus.append(xu)
                        ps_m = pp.tile([1, TT], F32, tag="pp", name="psmu")
                        ps_q = pp.tile([1, TT], F32, tag="pp", name="psqu")
                        for cc in range(NCC):
                            nc.tensor.matmul(ps_m[:], c_ones128[:], xus[cc][:],
                                             start=(cc == 0),
                                             stop=(cc == NCC - 1))
                        for cc in range(NCC):
                            xq = upf.tile([128, TT], BF16, tag="xu", name="xq")
                            nc.gpsimd.tensor_mul(xq[:], xus[cc][:], xus[cc][:])
                            nc.tensor.matmul(ps_q[:], c_ones128[:], xq[:],
                                             start=(cc == 0),
                                             stop=(cc == NCC - 1))
                        rm = upr.tile([1, TT], F32, tag="rmu", name="rmu")
                        rq = upr.tile([1, TT], F32, tag="rqu", name="rqu")
                        nc.vector.tensor_copy(rm[:], ps_m[:])
                        nc.vector.tensor_copy(rq[:], ps_q[:])
                        nc.scalar.dma_start(
                            rows_dram[tb, 0, tt * TT:(tt + 1) * TT], rm[:])
                        nc.scalar.dma_start(
                            rows_dram[tb, 1, tt * TT:(tt + 1) * TT], rq[:])

                emit_loads_stats(0)
                emit_post_stats(0)
                emit_f1(0)
                emit_p2rows(0, [0, 1, 2, 3, 4])
                emit_f2(0)
                emit_loads_stats(1)     # AR1 triggers before a2a_0
                emit_p2rows(1, [0, 1, 2, 3, 4])
                emit_rest(0)
                nc.gpsimd.collective_compute(
                    "AllToAll", OP.bypass, replica_groups=rg,
                    ins=[a2a_in[0][:].opt()], outs=[a2a_out[0][:].opt()])
                emit_post_stats(1)
                emit_f1(1)
                emit_f2(1)
                emit_rest(1)

            # ================= phase 2 =================
            with ExitStack() as st2:
                fc1p = st2.enter_context(tc.tile_pool(name="fc1p", bufs=1))
                fc2p = st2.enter_context(tc.tile_pool(name="fc2p", bufs=1))
                xtp = st2.enter_context(tc.tile_pool(name="xtp", bufs=12))
                hrp = st2.enter_context(tc.tile_pool(name="hrp", bufs=12))
                hbp = st2.enter_context(tc.tile_pool(name="hbp", bufs=12))
                hip = st2.enter_context(tc.tile_pool(name="hip", bufs=1))
                t1p = st2.enter_context(tc.tile_pool(name="t1p", bufs=4))
                hqp = st2.enter_context(tc.tile_pool(name="hqp", bufs=12))
                hxp = st2.enter_context(tc.tile_pool(name="hxp", bufs=12))
                hcp = st2.enter_context(tc.tile_pool(name="hcp", bufs=12))
                rwp = st2.enter_context(tc.tile_pool(name="rwp", bufs=2))
                rbp = st2.enter_context(tc.tile_pool(name="rbp", bufs=4))
                rw2 = st2.enter_context(tc.tile_pool(name="rw2", bufs=2))
                outp = st2.enter_context(tc.tile_pool(name="outp", bufs=2))
                ph = st2.enter_context(tc.tile_pool(name="ph", bufs=2, space="PSUM"))
                po = st2.enter_context(tc.tile_pool(name="po", bufs=2, space="PSUM"))
                pst = st2.enter_context(tc.tile_pool(name="pst", bufs=2, space="PSUM"))
                pbc = st2.enter_context(tc.tile_pool(name="pbc", bufs=2, space="PSUM"))

                c_fc1 = [fc1p.tile([128, HID], BF16, tag=f"fc1_{i}", name=f"cfc1_{i}")
                         for i in range(NCC)]
                for i in range(NCC):
                    nc.sync.dma_start(c_fc1[i][:], fc1m[i * 128:(i + 1) * 128, :])
                c_fc2 = [fc2p.tile([128, C], BF16, tag=f"fc2_{i}", name=f"cfc2_{i}")
                         for i in range(NMO)]
                for i in range(NMO):
                    nc.sync.dma_start(c_fc2[i][:], fc2w[i * 128:(i + 1) * 128, :])

                def p2_prep(tb, tt):
                    t0 = tt * TT
                    tg = tb * TSB + t0
                    rm = rwp.tile([1, TT], F32, tag="rm", name="rm")
                    rq = rwp.tile([1, TT], F32, tag="rq", name="rq")
                    vt = rwp.tile([1, TT], F32, tag="vt", name="vt")
                    nc.sync.dma_start(rm[:], rows_dram[tb, 0, t0:t0 + TT])
                    nc.sync.dma_start(rq[:], rows_dram[tb, 1, t0:t0 + TT])
                    nc.vector.tensor_scalar(out=rm[:], in0=rm[:],
                                            scalar1=1.0 / C, scalar2=None,
                                            op0=OP.mult)
                    nc.vector.tensor_scalar(out=rq[:], in0=rq[:],
                                            scalar1=1.0 / C, scalar2=None,
                                            op0=OP.mult)
                    nc.vector.tensor_mul(vt[:], rm[:], rm[:])
                    nc.vector.tensor_sub(rq[:], rq[:], vt[:])
                    nc.scalar.activation(out=rq[:], in_=rq[:],
                                         func=AF.Sqrt, bias=c_eps[:1])
                    nc.vector.reciprocal(rq[:], rq[:])
                    r1w = rwp.tile([1, TT], BF16, tag="r1w", name="r1w")
                    mr1w = rwp.tile([1, TT], BF16, tag="mr1w", name="mr1w")
                    nc.vector.tensor_copy(r1w[:], rq[:])
                    nc.vector.tensor_mul(rm[:], rm[:], rq[:])
                    nc.vector.tensor_copy(mr1w[:], rm[:])
                    r1b = rbp.tile([128, TT], BF16, tag="r1b", name="r1b")
                    mr1b = rbp.tile([128, TT], BF16, tag="mr1b", name="mr1b")
                    for rows, bt in ((r1w, r1b), (mr1w, mr1b)):
                        pb = pbc.tile([128, TT], F32, tag="pbc", name="pbt")
                        nc.tensor.matmul(pb[:], c_ones1[:], rows[:],
                                         start=True, stop=True)
                        nc.vector.tensor_copy(bt[:], pb[:])
                    htbs, hsqs = [], []
                    hxs, htrs = [], []
                    for cc in range(NCC):
                        xct = xtp.tile([128, TT], BF16, tag="xct", name="xct")
                        nc.gpsimd.dma_start(
                            xct[:], xc[cc * 128:(cc + 1) * 128, tg:tg + TT])
                        t1 = t1p.tile([128, TT], BF16, tag="t1", name="t1")
                        nc.vector.tensor_mul(t1[:], xct[:], r1b[:])
                        nc.vector.tensor_sub(t1[:], t1[:], mr1b[:])
                        nc.vector.tensor_scalar(out=t1[:], in0=t1[:],
                                                scalar1=c_g1f[:, cc:cc + 1],
                                                scalar2=c_be1f[:, cc:cc + 1],
                                                op0=OP.mult, op1=OP.add)
                        hx = hxp.tile([128, TT], BF16, tag="hx", name="hx")
                        nc.vector.tensor_add(hx[:], t1[:], xct[:])
                        hxs.append(hx)
                        htr = hrp.tile([128, TT], BF16, tag="htr", name="htr")
                        for (r0, sc, ci, n) in _recv_pieces(cc):
                            nc.gpsimd.dma_start(
                                htr[r0:r0 + n, :],
                                a2a_out[tb][sc, ci:ci + n, t0:t0 + TT])
                        htrs.append(htr)
                    for cc in range(NCC):
                        htb = hbp.tile([128, TT], BF16, tag="htb", name="htb")
                        nc.vector.tensor_add(htb[:], htrs[cc][:], hxs[cc][:])
                        htbs.append(htb)
                        hsq = hqp.tile([128, TT], BF16, tag="hsq", name="hsq")
                        nc.vector.tensor_mul(hsq[:], htb[:], htb[:])
                        hsqs.append(hsq)
                    return htbs, hsqs

                def p2_ln2(htbs, hsqs):
                    ps_s = pst.tile([1, TT], F32, tag="pst", name="pss")
                    ps_q = pst.tile([1, TT], F32, tag="pst", name="psq")
                    for cc in range(NCC):
                        nc.tensor.matmul(ps_s[:], c_ones128[:], htbs[cc][:],
                                         start=(cc == 0), stop=(cc == NCC - 1))
                    for cc in range(NCC):
                        nc.tensor.matmul(ps_q[:], c_ones128[:], hsqs[cc][:],
                                         start=(cc == 0), stop=(cc == NCC - 1))
                    m2r = rw2.tile([1, TT], F32, tag="m2r", name="m2r")
                    r2r = rw2.tile([1, TT], F32, tag="r2r", name="r2r")
                    vv = rw2.tile([1, TT], F32, tag="vv", name="vv")
                    nc.vector.tensor_scalar(out=m2r[:], in0=ps_s[:],
                                            scalar1=1.0 / C, scalar2=None,
                                            op0=OP.mult)
                    nc.vector.tensor_scalar(out=r2r[:], in0=ps_q[:],
                                            scalar1=1.0 / C, scalar2=None,
                                            op0=OP.mult)
                    nc.vector.tensor_mul(vv[:], m2r[:], m2r[:])
                    nc.vector.tensor_sub(r2r[:], r2r[:], vv[:])
                    nc.scalar.activation(out=r2r[:], in_=r2r[:],
                                         func=AF.Sqrt, bias=c_eps[:1])
                    nc.vector.reciprocal(r2r[:], r2r[:])
                    r2bf = rw2.tile([1, TT], BF16, tag="r2bf", name="r2bf")
                    m2rb = rw2.tile([1, TT], BF16, tag="m2rb", name="m2rb")
                    nc.vector.tensor_copy(r2bf[:], r2r[:])
                    nc.vector.tensor_mul(m2r[:], m2r[:], r2r[:])
                    nc.vector.tensor_copy(m2rb[:], m2r[:])
                    r2s = rbp.tile([128, TT], BF16, tag="r2s", name="r2s")
                    pb2 = pbc.tile([128, TT], F32, tag="pbc", name="pb2")
                    nc.tensor.matmul(pb2[:], c_ones1[:], r2bf[:],
                                     start=True, stop=True)
                    nc.vector.tensor_copy(r2s[:], pb2[:])
                    htcs = []
                    for cc in range(NCC):
                        htc = hcp.tile([128, TT], BF16, tag="htc", name="htc")
                        nc.vector.tensor_mul(htc[:], htbs[cc][:], r2s[:])
                        htcs.append(htc)
                    return htcs, m2rb

                def p2_compute(tb, tt, htbs, htcs, m2rb):
                    tg = tb * TSB + tt * TT
                    hid = hip.tile([128, NMO, TT], BF16, tag="hid", name="hid")
                    for mo in range(NMO):
                        php = ph.tile([128, TT], F32, tag="ph", name="php")
                        for cc in range(NCC):
                            nc.tensor.matmul(
                                php[:],
                                c_fc1[cc][:, mo * 128:(mo + 1) * 128],
                                htcs[cc][:],
                                start=(cc == 0), stop=False)
                        nc.tensor.matmul(php[:],
                                         c_uneg[:, mo * 128:(mo + 1) * 128],
                                         m2rb[:], start=False, stop=True)
                        nc.scalar.activation(out=hid[:, mo, :], in_=php[:],
                                             func=AF.Gelu,
                                             bias=c_gbias[:, mo:mo + 1])
                    for co in range(NCC):
                        pop = po.tile([128, TT], F32, tag="po", name="pop")
                        for ho in range(NMO):
                            nc.tensor.matmul(
                                pop[:],
                                c_fc2[ho][:, co * 128:(co + 1) * 128],
                                hid[:, ho, :],
                                start=(ho == 0), stop=(ho == NMO - 1))
                        of = outp.tile([128, TT], F32, tag="of", name="of")
                        nc.vector.scalar_tensor_tensor(
                            out=of[:], in0=pop[:],
                            scalar=c_fc2b[:, co:co + 1], in1=htbs[co][:],
                            op0=OP.add, op1=OP.add)
                        nc.sync.dma_start(
                            out[co * 128:(co + 1) * 128, tg:tg + TT], of[:])

                for tb in range(B):
                    cur_p = p2_prep(tb, 0)
                    cur_l = p2_ln2(*cur_p)
                    for tt in range(NT):
                        htbs, hsqs = cur_p
                        htcs, m2rb = cur_l
                        if tt + 1 < NT:
                            cur_p = p2_prep(tb, tt + 1)
                            cur_l = p2_ln2(*cur_p)
                        p2_compute(tb, tt, htbs, htcs, m2rb)
                    if tb == 0:
                        # emitted after tb=0 recv DMAs so its trigger wait
                        # (b=1 sends) never delays them on the gpsimd queue
                        nc.gpsimd.collective_compute(
                            "AllToAll", OP.bypass, replica_groups=rg,
                            ins=[a2a_in[1][:].opt()],
                            outs=[a2a_out[1][:].opt()])

    nc.compile()
    _CACHE["nc"] = nc
    return nc


def _host_prep(inputs):
    x = np.ascontiguousarray(np.asarray(inputs["x"], dtype=np.float32))
    g1 = np.asarray(inputs["g1"], np.float32); be1 = np.asarray(inputs["be1"], np.float32)
    g2 = np.asarray(inputs["g2"], np.float32); be2 = np.asarray(inputs["be2"], np.float32)
    w1 = np.asarray(inputs["w1"], np.float32); b1 = np.asarray(inputs["b1"], np.float32)
    w2 = np.asarray(inputs["w2"], np.float32); b2 = np.asarray(inputs["b2"], np.float32)
    fc1_w = np.asarray(inputs["fc1_w"], np.float32)
    fc1_b = np.asarray(inputs["fc1_b"], np.float32)
    fc2_w = np.asarray(inputs["fc2_w"], np.float32)
    fc2_b = np.asarray(inputs["fc2_b"], np.float32)

    dft = _dft_consts()
    xf = x.reshape(B, TOKB, C)
    fc1s = g2[:, None] * fc1_w
    fc1m_m = fc1s.astype(BF)                                     # (768, 3072)
    uneg_m = (-fc1s.sum(0, dtype=np.float64)).astype(BF)[None, :]
    gbias_v = (fc1_b + be2 @ fc1_w).astype(np.float32)           # (3072,)
    gbias_m = np.ascontiguousarray(gbias_v.reshape(NMO, 128).T)  # (128, 24)
    fc2b_m = np.ascontiguousarray(fc2_b.reshape(NCC, 128).T)
    g1f_m = np.ascontiguousarray(g1.reshape(NCC, 128).T)
    be1f_m = np.ascontiguousarray(be1.reshape(NCC, 128).T)
    ones1 = np.ones((1, 128), BF)
    ones128 = np.ones((128, 1), BF)

    in_maps = []
    for k in range(NCORES):
        ck = slice(k * BS, (k + 1) * BS)
        xw_k = np.ascontiguousarray(
            x[:, :, :, ck].transpose(2, 0, 1, 3)).astype(BF)     # [W,B,H,BS]
        xc_k = np.ascontiguousarray(
            np.concatenate([xf[0, k * TSB:(k + 1) * TSB],
                            xf[1, k * TSB:(k + 1) * TSB]], 0).T).astype(BF)
        g1k = g1[ck]
        w1r_k = w1[k, :, :, 0]; w1i_k = w1[k, :, :, 1]
        w1rp = (g1k[:, None] * w1r_k).astype(BF)
        w1ip = (g1k[:, None] * w1i_k).astype(BF)
        w1imp = (-(g1k[:, None] * w1i_k)).astype(BF)
        spike = (be1[ck] * SQN).astype(np.float64)
        b1sr_k = (w1r_k.T.astype(np.float64) @ spike).astype(np.float32)[:, None]
        b1si_k = (w1i_k.T.astype(np.float64) @ spike).astype(np.float32)[:, None]
        w2r_k = w2[k, :, :, 0]; w2i_k = w2[k, :, :, 1]
        w2p1_k = np.concatenate([w2r_k, w2i_k], 1).astype(BF)    # [96, 192]
        w2p2_k = np.concatenate([-w2i_k, w2r_k], 1).astype(BF)
        b2pk_k = np.concatenate([b2[k, :, 0], b2[k, :, 1]])[None, :].astype(BF)
        in_maps.append({
            "xw": xw_k, "xc": xc_k,
            **{n: dft[n] for n in ("fwp0", "fwp1", "f2a", "f2b", "iha", "ihb",
                                   "iwrt", "iwit")},
            "w1r": w1rp, "w1i": w1ip, "w1im": w1imp,
            "b1r": b1[k, :, 0:1].copy(), "b1i": b1[k, :, 1:2].copy(),
            "b1sr": b1sr_k, "b1si": b1si_k,
            "w2p1": w2p1_k, "w2p2": w2p2_k, "b2pk": b2pk_k,
            "fc1m": fc1m_m, "uneg": uneg_m, "gbias": gbias_m,
            "fc2w": fc2_w.astype(BF), "fc2b": fc2b_m,
            "g1f": g1f_m, "be1f": be1f_m,
            "ones1": ones1, "ones128": ones128,
        })
    return in_maps


def kernel(**inputs):
    nc = _build_nc()
    in_maps = _host_prep(inputs)
    res = run_bass_kernel_spmd(nc, in_maps, core_ids=list(range(NCORES)))
    full = np.empty((B, TOKB, C), np.float32)
    for j in range(NCORES):
        o = np.asarray(res.results[j]["out"], np.float32).T   # [4050, 768]
        full[0, j * TSB:(j + 1) * TSB] = o[:TSB]
        full[1, j * TSB:(j + 1) * TSB] = o[TSB:]
    return full.reshape(B, H, W, C)



# revision 11
# speedup vs baseline: 20920.4034x; 20920.4034x over previous
"""AFNO transformer block on 8 Trainium2 NeuronCores (bf16 + fp8 MLP).

Distribution:
  Phase 1 (channel-block sharded): core k owns channels [96k, 96k+96).
    LN1 partial stats -> per-batch AllReduce -> LN1 applied in place
    (gpsimd, 24-channel blocks) -> spectral path: F1 (W-DFT, flip), F2
    (H-DFT, flip, packed), block complex MLP (layer-2 bias via an
    augmented 97th contraction row), inverse H-DFT, inverse W-DFT
    emitted channel-major (loop over h) so the AllToAll sends are 8
    contiguous [96, 2025] DMAs instead of ~9k 360-byte bursts.
  Phase-2 LN1 row stats for all 10 token tiles are precomputed in the
    kernel-start window (PE + gpsimd idle while AllReduce 0 is in
    flight) and parked in DRAM.
  Phase 2 (token sharded): core j owns tokens [2025j, 2025(j+1)).
    h = filt + LN1(x) + x in bf16; LN row vectors broadcast to 128
    partitions via a DRAM round-trip DMA (zero-stride read); fc1/fc2 in
    fp8 e4m3 DoubleRow mode (K=256/matmul), descale via activation
    scale; fc2 bias folded in as a K=1 DoubleRow matmul; residual adds
    on gpsimd. When g1==1 and be1==0 (detected at build time) the LN1
    apply folds into x*(rstd+1) - m*rstd: 3 vector ops per channel
    chunk. Tile (0,0) is prepped during b=1's spectral phase; later
    tiles pipeline 1-deep (prep of k+1 fully inside compute of k).
"""
import math
import numpy as np
import ml_dtypes

import concourse.bass as bass
import concourse.mybir as mybir
import concourse.tile as tile
from concourse import bacc
from concourse.bass_utils import run_bass_kernel_spmd

F32 = mybir.dt.float32
BF16 = mybir.dt.bfloat16
FP8 = mybir.dt.float8e4
AF = mybir.ActivationFunctionType
OP = mybir.AluOpType
AX = mybir.AxisListType
PM = mybir.MatmulPerfMode

NCORES = 8
B, H, W, C = 2, 90, 180, 768
BS = 96            # channels per core / AFNO block size
KW = 46            # kept W-frequency modes
HID = 3072
LAM = 0.01
EPS = 1e-5
TOKB = H * W       # 16200 tokens per batch
TSB = TOKB // NCORES   # 2025 tokens per (core, batch)
TSH = 2 * TSB      # 4050 tokens per core
NM = KW * H        # 4140 modes per block
SQN = math.sqrt(H * W)
NCC = 6            # 768/128
NMO = 24           # 3072/128
TT = 405           # phase-2 token tile width
NT = TSB // TT     # 5 tiles per batch
M1CH = 460         # MLP1 chunk (4140 = 9*460)
BF = ml_dtypes.bfloat16
E4 = ml_dtypes.float8_e4m3
S1 = 64.0          # fc1 fp8 weight scale
S2 = 128.0         # fc2 fp8 weight scale


def _dft_consts():
    wv = np.arange(W, dtype=np.float64)[:, None]
    wf = np.arange(KW, dtype=np.float64)[None, :]
    ang = 2.0 * np.pi * wv * wf / W
    fwr = np.cos(ang) / math.sqrt(W)
    fwi = -np.sin(ang) / math.sqrt(W)
    fwpack = np.concatenate([fwr, fwi], axis=1)          # (180, 92)
    hv = np.arange(H, dtype=np.float64)[:, None]
    hf = np.arange(H, dtype=np.float64)[None, :]
    angh = 2.0 * np.pi * hv * hf / H
    fhc = np.cos(angh) / math.sqrt(H)
    fhs = np.sin(angh) / math.sqrt(H)
    fhsm = -fhs
    alpha = np.ones(KW); alpha[1:] = 2.0
    iwr = alpha[None, :] * np.cos(ang) / math.sqrt(W)    # (180, 46)
    iwi = -alpha[None, :] * np.sin(ang) / math.sqrt(W)
    iwrt = np.ascontiguousarray(iwr.T)                   # (46, 180)
    iwit = np.ascontiguousarray(iwi.T)
    c = {}
    c["fwp0"] = fwpack[:90]
    c["fwp1"] = fwpack[90:]
    c["f2a"] = np.concatenate([fhc, fhsm], axis=1)       # (90, 180)
    c["f2b"] = np.concatenate([fhs, fhc], axis=1)
    c["iha"] = np.concatenate([fhc, fhs], axis=1)
    c["ihb"] = np.concatenate([fhsm, fhc], axis=1)
    c["iwrt"] = iwrt
    c["iwit"] = iwit
    return {k: np.ascontiguousarray(v).astype(BF) for k, v in c.items()}


def _recv_pieces(cc):
    c0, out, r0 = cc * 128, [], 0
    while r0 < 128:
        s = (c0 + r0) // BS
        ci = (c0 + r0) % BS
        n = min(BS - ci, 128 - r0)
        out.append((r0, s, ci, n))
        r0 += n
    return out


_CACHE = {}


def _build_nc(simple_ln1):
    key = ("nc", simple_ln1)
    if key in _CACHE:
        return _CACHE[key]
    nc = bacc.Bacc("TRN2", target_bir_lowering=False, debug=False,
                   num_devices=NCORES)

    def g(n, s, dt=BF16):
        return nc.dram_tensor(n, s, dt, kind="ExternalInput")

    xw = g("xw", [W, B, H, BS])
    xc = g("xc", [C, TSH])
    fwp0 = g("fwp0", [90, 92]); fwp1 = g("fwp1", [90, 92])
    f2a = g("f2a", [90, 180]); f2b = g("f2b", [90, 180])
    iha = g("iha", [90, 180]); ihb = g("ihb", [90, 180])
    iwrt = g("iwrt", [KW, W]); iwit = g("iwit", [KW, W])
    w1r = g("w1r", [BS, BS]); w1i = g("w1i", [BS, BS]); w1im = g("w1im", [BS, BS])
    b1r = g("b1r", [BS, 1], F32); b1i = g("b1i", [BS, 1], F32)
    b1sr = g("b1sr", [BS, 1], F32); b1si = g("b1si", [BS, 1], F32)
    w2p1 = g("w2p1", [BS + 1, 192]); w2p2 = g("w2p2", [BS + 1, 192])
    fc1dr = g("fc1dr", [3, 128, 2, HID], FP8)
    fc2dr = g("fc2dr", [12, 128, 2, C], FP8)
    fc2bias = g("fc2bias", [1, 2, C], FP8)
    onesdr = g("onesdr", [1, 2, TT], FP8)
    gbias = g("gbias", [128, NMO], F32)
    g1f = g("g1f", [128, NCC], F32); be1f = g("be1f", [128, NCC], F32)
    ones128 = g("ones128", [128, 1])
    o1row = g("o1row", [2, NM])

    out = nc.dram_tensor("out", [C, TSH], F32, kind="ExternalOutput")
    rg = [list(range(NCORES))]

    tiles = [(b_, t_) for b_ in range(B) for t_ in range(NT)]
    NTILES = len(tiles)

    from contextlib import ExitStack
    with tile.TileContext(nc) as tc:
        with ExitStack() as st0:
            cp = st0.enter_context(tc.tile_pool(name="const", bufs=1))
            dram = st0.enter_context(tc.tile_pool(name="dram", bufs=1,
                                                  space="DRAM"))
            dstg = st0.enter_context(tc.tile_pool(name="dstg", bufs=4,
                                                  space="DRAM"))
            # early-prep pools (tile k=0 + row-stat precompute)
            exct = st0.enter_context(tc.tile_pool(name="exct", bufs=2))
            ehtr = st0.enter_context(tc.tile_pool(name="ehtr", bufs=1))
            ehtb = st0.enter_context(tc.tile_pool(name="ehtb", bufs=1))
            escr = st0.enter_context(tc.tile_pool(name="escr", bufs=1))
            ebcp = st0.enter_context(tc.tile_pool(name="ebcp", bufs=1))
            ehtc = st0.enter_context(tc.tile_pool(name="ehtc", bufs=1))
            erw = st0.enter_context(tc.tile_pool(name="erw", bufs=2))
            pln = st0.enter_context(tc.tile_pool(name="pln", bufs=2,
                                                 space="PSUM"))

            def cl(t, shape, dt=BF16):
                nm = f"c_{t.name}"
                s = cp.tile(shape, dt, name=nm, tag=nm)
                nc.sync.dma_start(s[:], t[:])
                return s

            c_fwp0 = cl(fwp0, [90, 92]); c_fwp1 = cl(fwp1, [90, 92])
            c_f2a = cl(f2a, [90, 180]); c_f2b = cl(f2b, [90, 180])
            c_iha = cl(iha, [90, 180]); c_ihb = cl(ihb, [90, 180])
            c_iwrt = cl(iwrt, [KW, W]); c_iwit = cl(iwit, [KW, W])
            c_w1r = cl(w1r, [BS, BS]); c_w1i = cl(w1i, [BS, BS])
            c_w1im = cl(w1im, [BS, BS])
            c_b1r = cl(b1r, [BS, 1], F32); c_b1i = cl(b1i, [BS, 1], F32)
            c_b1sr = cl(b1sr, [BS, 1], F32); c_b1si = cl(b1si, [BS, 1], F32)
            c_w2p1 = cl(w2p1, [BS + 1, 192]); c_w2p2 = cl(w2p2, [BS + 1, 192])
            c_gbias = cl(gbias, [128, NMO], F32)
            c_g1f = cl(g1f, [128, NCC], F32); c_be1f = cl(be1f, [128, NCC], F32)
            c_ones128 = cl(ones128, [128, 1])
            c_fc2bias = cl(fc2bias, [1, 2, C], FP8)
            c_onesdr = cl(onesdr, [1, 2, TT], FP8)
            c_eps = cp.tile([128, 1], F32, name="c_eps")
            nc.vector.memset(c_eps[:], EPS)

            st_in = [dram.tile([2, TOKB], F32, name=f"st_in{b_}") for b_ in range(B)]
            st_out = [dram.tile([2, TOKB], F32, name=f"st_out{b_}") for b_ in range(B)]
            a2a_in = [dram.tile([NCORES, BS, TSB], BF16, name=f"a2a_in{b_}")
                      for b_ in range(B)]
            a2a_out = [dram.tile([NCORES, BS, TSB], BF16, name=f"a2a_out{b_}")
                       for b_ in range(B)]
            rows_dram = dram.tile([B, 2, TSB], F32, name="rows_dram")
            stg_e0 = dram.tile([2, TT], BF16, name="stg_e0")

            # ---------- shared helpers ----------
            def bcast_read(stg, pool, tag):
                bc = pool.tile([128, 2, TT], BF16, tag=tag, name=tag)
                nc.sync.dma_start(
                    bc[:], bass.AP(tensor=stg[:].tensor, offset=stg[:].offset,
                                   ap=[[0, 128], [TT, 2], [1, TT]]))
                return bc

            def stage_write(a, b_, stg):
                nc.sync.dma_start(stg[0:1, :], a[:])
                nc.sync.dma_start(stg[1:2, :], b_[:])

            def row_math(rm, rq, pool, plus1=False):
                """raw sums rm, sq-sums rq [1,TT] f32 -> bf16 row pair.

                Returns (rstd [+1 if plus1], mean*rstd)."""
                vt = pool.tile([1, TT], F32, tag="vt", name="vt")
                nc.vector.tensor_scalar(out=rm[:], in0=rm[:], scalar1=1.0 / C,
                                        scalar2=None, op0=OP.mult)
                nc.vector.tensor_scalar(out=rq[:], in0=rq[:], scalar1=1.0 / C,
                                        scalar2=None, op0=OP.mult)
                nc.vector.tensor_mul(vt[:], rm[:], rm[:])
                nc.vector.tensor_sub(rq[:], rq[:], vt[:])
                nc.scalar.activation(out=rq[:], in_=rq[:], func=AF.Sqrt,
                                     bias=c_eps[:1])
                nc.vector.reciprocal(rq[:], rq[:])
                r1w = pool.tile([1, TT], BF16, tag="r1w", name="r1w")
                mr1w = pool.tile([1, TT], BF16, tag="mr1w", name="mr1w")
                nc.vector.tensor_mul(rm[:], rm[:], rq[:])
                nc.vector.tensor_copy(mr1w[:], rm[:])
                if plus1:
                    nc.vector.tensor_scalar(out=r1w[:], in0=rq[:],
                                            scalar1=1.0, scalar2=None,
                                            op0=OP.add)
                else:
                    nc.vector.tensor_copy(r1w[:], rq[:])
                return r1w, mr1w

            def t1_chain(xall, htrall, rbc, p_scr, p_htb):
                """htb = filt + LN1(x) + x per channel chunk; LN2 sums."""
                t3 = p_scr.tile([128, NCC, TT], BF16, tag="big3", name="t3")
                htb = p_htb.tile([128, NCC, TT], BF16, tag="htb", name="htb")
                for cc in range(NCC):
                    ts = t3[:, cc, :]
                    nc.vector.tensor_mul(ts, xall[:, cc, :], rbc[:, 0, :])
                    nc.vector.tensor_sub(ts, ts, rbc[:, 1, :])
                    if not simple_ln1:
                        nc.vector.tensor_scalar(out=ts, in0=ts,
                                                scalar1=c_g1f[:, cc:cc + 1],
                                                scalar2=c_be1f[:, cc:cc + 1],
                                                op0=OP.mult, op1=OP.add)
                        nc.vector.tensor_add(ts, ts, xall[:, cc, :])
                    nc.vector.tensor_add(htb[:, cc, :], htrall[:, cc, :], ts)
                hq3 = p_scr.tile([128, NCC, TT], BF16, tag="big3", name="hq3")
                nc.scalar.activation(out=hq3[:], in_=htb[:], func=AF.Square)
                hs = p_scr.tile([128, TT], BF16, tag="hs", name="hs")
                hq = p_scr.tile([128, TT], BF16, tag="hq", name="hq")
                with nc.allow_low_precision(reason="bf16 stat partials"):
                    nc.vector.tensor_add(hs[:], htb[:, 0, :], htb[:, 1, :])
                    for cc in range(2, NCC):
                        nc.vector.tensor_add(hs[:], hs[:], htb[:, cc, :])
                    nc.vector.tensor_add(hq[:], hq3[:, 0, :], hq3[:, 1, :])
                    for cc in range(2, NCC):
                        nc.vector.tensor_add(hq[:], hq[:], hq3[:, cc, :])
                return htb, hs, hq

            def ln2_mm(hs, hq, p_ps):
                pss = p_ps.tile([1, TT], F32, tag="pst", name="pss")
                psq = p_ps.tile([1, TT], F32, tag="pst", name="psq")
                nc.tensor.matmul(pss[:], c_ones128[:], hs[:], start=True,
                                 stop=True)
                nc.tensor.matmul(psq[:], c_ones128[:], hq[:], start=True,
                                 stop=True)
                return pss, psq

            def ln2_fin(pss, psq, p_rw, stg, p_bc, tag):
                m2 = p_rw.tile([1, TT], F32, tag="m2", name="m2")
                r2 = p_rw.tile([1, TT], F32, tag="r2", name="r2")
                nc.vector.tensor_copy(m2[:], pss[:])
                nc.vector.tensor_copy(r2[:], psq[:])
                r2b, m2b = row_math(m2, r2, p_rw)
                stage_write(r2b, m2b, stg)
                return bcast_read(stg, p_bc, tag)

            def htc_make(htb, cbc, p_scr, p_htc):
                tmp3 = p_scr.tile([128, NCC, TT], BF16, tag="big3",
                                  name="tmp3")
                htcp = p_htc.tile([128, NCC, TT], FP8, tag="htcp",
                                  name="htcp")
                for cc in range(NCC):
                    nc.vector.tensor_mul(tmp3[:, cc, :], htb[:, cc, :],
                                         cbc[:, 0, :])
                    nc.vector.tensor_sub(htcp[:, cc, :], tmp3[:, cc, :],
                                         cbc[:, 1, :])
                return htcp

            def load_xall(pool, tg):
                xall = pool.tile([128, NCC, TT], BF16, tag="xall",
                                 name="xall")
                for cc in range(NCC):
                    nc.sync.dma_start(
                        xall[:, cc, :],
                        xc[cc * 128:(cc + 1) * 128, tg:tg + TT])
                return xall

            def load_htr(pool, tb, t0):
                htrall = pool.tile([128, NCC, TT], BF16, tag="htr",
                                   name="htr")
                for cc in range(NCC):
                    for (r0, sc, ci, n) in _recv_pieces(cc):
                        nc.sync.dma_start(
                            htrall[r0:r0 + n, cc, :],
                            a2a_out[tb][sc, ci:ci + n, t0:t0 + TT])
                return htrall

            # ================= phase 1 =================
            with ExitStack() as st1:
                sqp = st1.enter_context(tc.tile_pool(name="sqp", bufs=2))
                stp = st1.enter_context(tc.tile_pool(name="stats", bufs=1))
                zp = st1.enter_context(tc.tile_pool(name="zp", bufs=2))
                clp = st1.enter_context(tc.tile_pool(name="clp", bufs=2))
                bigp = st1.enter_context(tc.tile_pool(name="bigp", bufs=2))
                s2p = st1.enter_context(tc.tile_pool(name="s2p", bufs=1))
                pp = st1.enter_context(tc.tile_pool(name="psum1", bufs=6,
                                                    space="PSUM"))
                zhs = {}

                def stk(t, kind):
                    return bass.AP(tensor=t[:].tensor,
                                   offset=t[:].offset + kind * TOKB,
                                   ap=[[90, 90], [8100, 2], [1, 90]])

                def emit_loads_stats(b):
                    """Load z (bf16), partial LN1 stats, AllReduce trigger."""
                    eng = nc.vector
                    zh = []
                    for wc in range(2):
                        zt = zp.tile([90, H, BS], BF16, tag="zh",
                                     name=f"zh{b}{wc}")
                        eng_ld = nc.scalar if b == 0 else nc.sync
                        eng_ld.dma_start(
                            zt[:], xw[wc * 90:(wc + 1) * 90, b, :, :])
                        zh.append(zt)
                    zhs[b] = zh
                    s_sum = stp.tile([90, 2, H], F32, tag="ssum")
                    s_sq = stp.tile([90, 2, H], F32, tag="ssq")
                    s_t = stp.tile([90, H], F32, tag="st_t")
                    zhs[b, "sum"] = s_sum
                    zhs[b, "sq"] = s_sq
                    for wc in range(2):
                        zt = zh[wc]
                        eng.reduce_sum(s_sum[:, wc, :], zt[:], axis=AX.X)
                        for blk in range(4):
                            sqt = sqp.tile([90, H, 24], BF16, tag="sqt")
                            zsl = zt[:, :, blk * 24:(blk + 1) * 24]
                            if b == 0:
                                nc.scalar.activation(out=sqt[:], in_=zsl,
                                                     func=AF.Square)
                            else:
                                nc.gpsimd.tensor_mul(sqt[:], zsl, zsl)
                            if blk == 0:
                                eng.reduce_sum(s_sq[:, wc, :], sqt[:], axis=AX.X)
                            else:
                                eng.reduce_sum(s_t[:], sqt[:], axis=AX.X)
                                eng.tensor_add(s_sq[:, wc, :], s_sq[:, wc, :],
                                               s_t[:])
                    nc.sync.dma_start(stk(st_in[b], 0), s_sum[:])
                    nc.sync.dma_start(stk(st_in[b], 1), s_sq[:])
                    nc.gpsimd.collective_compute(
                        "AllReduce", OP.add, replica_groups=rg,
                        ins=[st_in[b][:].opt()], outs=[st_out[b][:].opt()])

                def emit_post_stats(b):
                    """st recv, m/r, LN1 apply in place (gpsimd)."""
                    s_sum, s_sq = zhs[b, "sum"], zhs[b, "sq"]
                    nc.sync.dma_start(s_sum[:], stk(st_out[b], 0))
                    nc.sync.dma_start(s_sq[:], stk(st_out[b], 1))
                    s_m = stp.tile([90, 2, H], F32, tag="sm")
                    s_r = stp.tile([90, 2, H], F32, tag="sr")
                    s_v = stp.tile([90, 2, H], F32, tag="sv")
                    nc.vector.tensor_scalar(out=s_m[:], in0=s_sum[:],
                                            scalar1=1.0 / C, scalar2=None,
                                            op0=OP.mult)
                    nc.vector.tensor_scalar(out=s_r[:], in0=s_sq[:],
                                            scalar1=1.0 / C, scalar2=None,
                                            op0=OP.mult)
                    nc.vector.tensor_mul(s_v[:], s_m[:], s_m[:])
                    nc.vector.tensor_sub(s_r[:], s_r[:], s_v[:])
                    nc.scalar.activation(out=s_r[:], in_=s_r[:],
                                         func=AF.Sqrt, bias=c_eps[:90])
                    nc.vector.reciprocal(s_r[:], s_r[:])
                    s_rb = stp.tile([90, 2, H], BF16, tag="srb")
                    s_mrb = stp.tile([90, 2, H], BF16, tag="smrb")
                    nc.vector.tensor_copy(s_rb[:], s_r[:])
                    nc.vector.tensor_mul(s_v[:], s_m[:], s_r[:])
                    nc.vector.tensor_copy(s_mrb[:], s_v[:])
                    zhs[b, "smrb"] = s_mrb

                    def bc(t, wc, n):
                        a = t[:, wc, :]
                        return bass.AP(tensor=a.tensor, offset=a.offset,
                                       ap=[list(a.ap[0]), [1, H], [0, n]])
                    for c0 in range(0, BS, 24):
                        for wc in range(2):
                            zv = zhs[b][wc][:, :, c0:c0 + 24]
                            nc.gpsimd.tensor_mul(zv, zv, bc(s_rb, wc, 24))

                def emit_f1(b):
                    zh = zhs[b]
                    s_mrb = zhs[b, "smrb"]
                    fmr = stp.tile([90, 92], F32, tag="fmr", name="fmr")
                    pfm = pp.tile([90, 92], F32, tag="pp", name="psfmr")
                    nc.tensor.matmul(pfm[:], s_mrb[:, 0, :], c_fwp0[:],
                                     start=True, stop=False)
                    nc.tensor.matmul(pfm[:], s_mrb[:, 1, :], c_fwp1[:],
                                     start=False, stop=True)
                    nc.vector.tensor_copy(fmr[:], pfm[:])
                    yb = bigp.tile([90, BS, 92], BF16, tag="big", name=f"yb{b}")
                    zhs[b, "yb"] = yb
                    for gi, c0 in enumerate(range(0, BS, 4)):
                        pf = pp.tile([90, 4 * 92], F32, tag="pp", name="psf1")
                        for ci in range(4):
                            c = c0 + ci
                            nc.tensor.matmul(pf[:, ci * 92:(ci + 1) * 92],
                                             zh[0][:, :, c], c_fwp0[:],
                                             start=True, stop=False)
                            nc.tensor.matmul(pf[:, ci * 92:(ci + 1) * 92],
                                             zh[1][:, :, c], c_fwp1[:],
                                             start=False, stop=True)
                        dst = yb[:, c0:c0 + 4, :]
                        src = pf[:].rearrange("p (a b) -> p a b", a=4)
                        fa = fmr[:]
                        fbc = bass.AP(tensor=fa.tensor, offset=fa.offset,
                                      ap=[list(fa.ap[0]), [0, 4], [1, 92]])
                        nc.vector.tensor_tensor(out=dst, in0=src, in1=fbc,
                                                op=OP.subtract)

                def emit_f2(b):
                    yb = zhs[b, "yb"]
                    zb = bigp.tile([BS, 2, KW, H], BF16, tag="big",
                                   name=f"zb{b}")
                    zhs[b, "zb"] = zb
                    for wf in range(KW):
                        pz = pp.tile([BS, 180], F32, tag="pp", name="psf2")
                        nc.tensor.matmul(pz[:], yb[:, :, wf], c_f2a[:],
                                         start=True, stop=False)
                        nc.tensor.matmul(pz[:], yb[:, :, 46 + wf], c_f2b[:],
                                         start=False, stop=True)
                        if wf % 2 == 0:
                            nc.scalar.activation(
                                out=zb[:, :, wf, :],
                                in_=pz[:].rearrange("p (a b) -> p a b", a=2),
                                func=AF.Copy)
                        else:
                            nc.vector.tensor_copy(
                                zb[:, :, wf, :],
                                pz[:].rearrange("p (a b) -> p a b", a=2))

                def emit_mlp(b):
                    zb = zhs[b, "zb"]
                    o1 = bigp.tile([BS + 1, 2, NM], BF16, tag="big",
                                   name=f"o1{b}")
                    nc.sync.dma_start(o1[96:97, :, :], o1row[:])
                    zr_f = zb[:, 0].rearrange("p a b -> p (a b)")
                    zi_f = zb[:, 1].rearrange("p a b -> p (a b)")
                    for ch in range(9):
                        n0 = ch * M1CH
                        zr_s = zr_f[:, n0:n0 + M1CH]
                        zi_s = zi_f[:, n0:n0 + M1CH]
                        por = pp.tile([BS, M1CH], F32, tag="pp", name="pso1r")
                        nc.tensor.matmul(por[:], c_w1r[:], zr_s,
                                         start=True, stop=False)
                        nc.tensor.matmul(por[:], c_w1im[:], zi_s,
                                         start=False, stop=True)
                        poi = pp.tile([BS, M1CH], F32, tag="pp", name="pso1i")
                        nc.tensor.matmul(poi[:], c_w1i[:], zr_s,
                                         start=True, stop=False)
                        nc.tensor.matmul(poi[:], c_w1r[:], zi_s,
                                         start=False, stop=True)
                        if ch == 0:
                            # be1 spike contribution on mode (0,0) only
                            nc.vector.tensor_scalar(out=por[:, 0:1],
                                                    in0=por[:, 0:1],
                                                    scalar1=c_b1sr[:],
                                                    scalar2=None, op0=OP.add)
                            nc.vector.tensor_scalar(out=poi[:, 0:1],
                                                    in0=poi[:, 0:1],
                                                    scalar1=c_b1si[:],
                                                    scalar2=None, op0=OP.add)
                        nc.scalar.activation(out=o1[0:BS, 0, n0:n0 + M1CH],
                                             in_=por[:], func=AF.Relu,
                                             bias=c_b1r[:])
                        nc.scalar.activation(out=o1[0:BS, 1, n0:n0 + M1CH],
                                             in_=poi[:], func=AF.Relu,
                                             bias=c_b1i[:])

                    o2 = bigp.tile([H, 2, KW, BS], BF16, tag="big",
                                   name=f"o2{b}")
                    zhs[b, "o2"] = o2
                    for wf in range(KW):
                        lr = o1[:, 0, wf * H:(wf + 1) * H]
                        li = o1[:, 1, wf * H:(wf + 1) * H]
                        pm = pp.tile([H, 192], F32, tag="pp", name="pso2")
                        nc.tensor.matmul(pm[:], lr, c_w2p1[:],
                                         start=True, stop=False)
                        nc.tensor.matmul(pm[:], li, c_w2p2[:],
                                         start=False, stop=True)
                        clip = clp.tile([H, 192], F32, tag="clip")
                        nc.vector.tensor_scalar(out=clip[:], in0=pm[:],
                                                scalar1=-LAM, scalar2=LAM,
                                                op0=OP.max, op1=OP.min)
                        nc.vector.tensor_tensor(
                            out=o2[:, :, wf, :],
                            in0=pm[:].rearrange("p (a b) -> p a b", a=2),
                            in1=clip[:].rearrange("p (a b) -> p a b", a=2),
                            op=OP.subtract)

                def emit_ih(b):
                    o2 = zhs[b, "o2"]
                    u2 = bigp.tile([KW, 2, BS, H], BF16, tag="big",
                                   name=f"u2{b}")
                    zhs[b, "u2"] = u2
                    for c in range(BS):
                        lr = o2[:, 0, :, c]
                        li = o2[:, 1, :, c]
                        pu = pp.tile([KW, 180], F32, tag="pp", name="psu")
                        nc.tensor.matmul(pu[:], lr, c_iha[:],
                                         start=True, stop=False)
                        nc.tensor.matmul(pu[:], li, c_ihb[:],
                                         start=False, stop=True)
                        dst = u2[:, :, c, :]
                        src = pu[:].rearrange("p (a b) -> p a b", a=2)
                        if c % 2 == 0:
                            nc.scalar.activation(out=dst, in_=src, func=AF.Copy)
                        else:
                            nc.vector.tensor_copy(dst, src)

                def emit_iw(b):
                    """inverse W-DFT, channel-major; contiguous a2a sends."""
                    u2 = zhs[b, "u2"]
                    s2c = s2p.tile([BS, H, W], BF16, tag="s2", name=f"s2{b}")
                    for h in range(H):
                        pf = pp.tile([BS, W], F32, tag="pp", name="psw")
                        nc.tensor.matmul(pf[:], u2[:, 0, :, h], c_iwrt[:],
                                         start=True, stop=False)
                        nc.tensor.matmul(pf[:], u2[:, 1, :, h], c_iwit[:],
                                         start=False, stop=True)
                        if h % 2 == 0:
                            nc.scalar.activation(out=s2c[:, h, :], in_=pf[:],
                                                 func=AF.Copy)
                        else:
                            nc.vector.tensor_copy(s2c[:, h, :], pf[:])
                    flat = s2c[:].rearrange("p a b -> p (a b)")
                    for j in range(NCORES):
                        dst = bass.AP(
                            tensor=a2a_in[b][:].tensor,
                            offset=a2a_in[b][:].offset + j * BS * TSB,
                            ap=[[TSB, BS], [1, TSB]])
                        nc.scalar.dma_start(dst, flat[:, j * TSB:(j + 1) * TSB])

                def p2rows_all():
                    """LN1 row stats for every phase-2 tile, parked in DRAM.

                    Runs in the kernel-start window while AllReduce 0 is
                    in flight (PE/gpsimd otherwise idle)."""
                    for kk, (tb, tt) in enumerate(tiles):
                        tg = tb * TSB + tt * TT
                        xall = load_xall(exct, tg)
                        xq3 = escr.tile([128, NCC, TT], BF16, tag="big3",
                                        name="xq3")
                        nc.gpsimd.tensor_mul(xq3[:], xall[:], xall[:])
                        ps_m = pln.tile([1, TT], F32, tag="pst", name="psm")
                        ps_q = pln.tile([1, TT], F32, tag="pst", name="psq")
                        for cc in range(NCC):
                            nc.tensor.matmul(ps_m[:], c_ones128[:],
                                             xall[:, cc, :],
                                             start=(cc == 0),
                                             stop=(cc == NCC - 1))
                        for cc in range(NCC):
                            nc.tensor.matmul(ps_q[:], c_ones128[:],
                                             xq3[:, cc, :],
                                             start=(cc == 0),
                                             stop=(cc == NCC - 1))
                        rm = erw.tile([1, TT], F32, tag="rmu", name="rmu")
                        rq = erw.tile([1, TT], F32, tag="rqu", name="rqu")
                        nc.vector.tensor_copy(rm[:], ps_m[:])
                        nc.vector.tensor_copy(rq[:], ps_q[:])
                        if kk == 0:
                            r1w, mr1w = row_math(rm, rq, erw,
                                                 plus1=simple_ln1)
                            stage_write(r1w, mr1w, stg_e0)
                        else:
                            t0 = tt * TT
                            nc.scalar.dma_start(
                                rows_dram[tb, 0, t0:t0 + TT], rm[:])
                            nc.scalar.dma_start(
                                rows_dram[tb, 1, t0:t0 + TT], rq[:])

                # ---- phase-1 emission schedule ----
                emit_loads_stats(0)
                p2rows_all()
                emit_post_stats(0)
                emit_f1(0)
                emit_f2(0)
                emit_loads_stats(1)     # AR1 trigger
                emit_mlp(0)
                emit_ih(0)
                emit_post_stats(1)      # apply b=1 on gpsimd, pre-trigger
                emit_iw(0)
                nc.gpsimd.collective_compute(
                    "AllToAll", OP.bypass, replica_groups=rg,
                    ins=[a2a_in[0][:].opt()], outs=[a2a_out[0][:].opt()])
                emit_f1(1)
                emit_f2(1)
                exall0 = load_xall(exct, 0)
                ehtr0 = load_htr(ehtr, 0, 0)
                ebc0 = bcast_read(stg_e0, ebcp, "ebc")
                emit_mlp(1)
                ehtb0, ehs0, ehq0 = t1_chain(exall0, ehtr0, ebc0, escr, ehtb)
                emit_ih(1)
                pss0, psq0 = ln2_mm(ehs0, ehq0, pln)
                stgl0 = dstg.tile([2, TT], BF16, tag="stgl", name="stgl0")
                cbc0 = ln2_fin(pss0, psq0, erw, stgl0, ebcp, "ecbc")
                htcp0 = htc_make(ehtb0, cbc0, escr, ehtc)
                emit_iw(1)
                nc.gpsimd.collective_compute(
                    "AllToAll", OP.bypass, replica_groups=rg,
                    ins=[a2a_in[1][:].opt()], outs=[a2a_out[1][:].opt()])

            # ================= phase 2 =================
            with ExitStack() as st2:
                fc1p = st2.enter_context(tc.tile_pool(name="fc1p", bufs=1))
                fc2p = st2.enter_context(tc.tile_pool(name="fc2p", bufs=1))
                xtp = st2.enter_context(tc.tile_pool(name="xtp", bufs=2))
                hrp = st2.enter_context(tc.tile_pool(name="hrp", bufs=2))
                hbp = st2.enter_context(tc.tile_pool(name="hbp", bufs=2))
                hip = st2.enter_context(tc.tile_pool(name="hip", bufs=1))
                scp = st2.enter_context(tc.tile_pool(name="scp", bufs=1))
                hcp = st2.enter_context(tc.tile_pool(name="hcp", bufs=2))
                bcp = st2.enter_context(tc.tile_pool(name="bcp", bufs=2))
                rwp = st2.enter_context(tc.tile_pool(name="rwp", bufs=1))
                outp = st2.enter_context(tc.tile_pool(name="outp", bufs=2))
                pst = st2.enter_context(tc.tile_pool(name="pst", bufs=2,
                                                     space="PSUM"))
                pc = st2.enter_context(tc.tile_pool(name="pc", bufs=4,
                                                    space="PSUM"))

                c_fc1 = [fc1p.tile([128, 2, HID], FP8, tag=f"fc1_{i}",
                                   name=f"cfc1_{i}") for i in range(3)]
                for i in range(3):
                    nc.sync.dma_start(c_fc1[i][:], fc1dr[i])
                c_fc2 = [fc2p.tile([128, 2, C], FP8, tag=f"fc2_{i}",
                                   name=f"cfc2_{i}") for i in range(12)]
                for i in range(12):
                    nc.sync.dma_start(c_fc2[i][:], fc2dr[i])

                def pA_load(k):
                    tb, tt = tiles[k]
                    return (load_xall(xtp, tb * TSB + tt * TT),
                            load_htr(hrp, tb, tt * TT))

                def pA_rest(k, xall, htrall):
                    tb, tt = tiles[k]
                    rm = rwp.tile([1, TT], F32, tag="rm", name="rm")
                    rq = rwp.tile([1, TT], F32, tag="rq", name="rq")
                    t0 = tt * TT
                    nc.sync.dma_start(rm[:], rows_dram[tb, 0, t0:t0 + TT])
                    nc.sync.dma_start(rq[:], rows_dram[tb, 1, t0:t0 + TT])
                    r1w, mr1w = row_math(rm, rq, rwp, plus1=simple_ln1)
                    stg = dstg.tile([2, TT], BF16, tag="stgr", name="stgr")
                    stage_write(r1w, mr1w, stg)
                    rbc = bcast_read(stg, bcp, "rbc")
                    htb, hs, hq = t1_chain(xall, htrall, rbc, scp, hbp)
                    pss, psq2 = ln2_mm(hs, hq, pst)
                    return htb, pss, psq2

                def pB(k, htb, pss, psq):
                    stg = dstg.tile([2, TT], BF16, tag="stgl", name="stgl")
                    cbc = ln2_fin(pss, psq, rwp, stg, bcp, "cbc")
                    htcp = htc_make(htb, cbc, scp, hcp)
                    return htb, htcp

                def pC(k, htb, htcp):
                    tb, tt = tiles[k]
                    tg = tb * TSB + tt * TT
                    hid = hip.tile([128, NMO, TT], FP8, tag="hid", name="hid")
                    for mo in range(NMO):
                        php = pc.tile([128, TT], F32, tag="pc", name="php")
                        for ccp in range(3):
                            nc.tensor.matmul(
                                php[:],
                                c_fc1[ccp][:, :, mo * 128:(mo + 1) * 128],
                                htcp[:, 2 * ccp:2 * ccp + 2, :],
                                start=(ccp == 0), stop=(ccp == 2),
                                perf_mode=PM.DoubleRow)
                        nc.scalar.activation(out=hid[:, mo, :], in_=php[:],
                                             func=AF.Gelu,
                                             bias=c_gbias[:, mo:mo + 1],
                                             scale=1.0 / S1)
                    for co in range(NCC):
                        pop = pc.tile([128, TT], F32, tag="pc", name="pop")
                        for ho in range(12):
                            nc.tensor.matmul(
                                pop[:],
                                c_fc2[ho][:, :, co * 128:(co + 1) * 128],
                                hid[:, 2 * ho:2 * ho + 2, :],
                                start=(ho == 0), stop=False,
                                perf_mode=PM.DoubleRow)
                        nc.tensor.matmul(
                            pop[:], c_fc2bias[:, :, co * 128:(co + 1) * 128],
                            c_onesdr[:], start=False, stop=True,
                            perf_mode=PM.DoubleRow)
                        oft = outp.tile([128, TT], BF16, tag="oft", name="oft")
                        nc.scalar.activation(out=oft[:], in_=pop[:],
                                             func=AF.Copy, scale=1.0 / S2)
                        of = outp.tile([128, TT], F32, tag="of", name="of")
                        nc.gpsimd.tensor_add(of[:], oft[:], htb[:, co, :])
                        nc.sync.dma_start(
                            out[co * 128:(co + 1) * 128, tg:tg + TT], of[:])

                stB = {0: (ehtb0, htcp0)}
                Ld = {1: pA_load(1)}
                for k in range(NTILES):
                    if k + 2 < NTILES:
                        Ld[k + 2] = pA_load(k + 2)
                    pC(k, *stB[k])
                    if k + 1 < NTILES:
                        stA = pA_rest(k + 1, *Ld[k + 1])
                        stB[k + 1] = pB(k + 1, *stA)

    nc.compile()
    _CACHE[key] = nc
    return nc


def _host_prep(inputs):
    x = np.ascontiguousarray(np.asarray(inputs["x"], dtype=np.float32))
    g1 = np.asarray(inputs["g1"], np.float32); be1 = np.asarray(inputs["be1"], np.float32)
    g2 = np.asarray(inputs["g2"], np.float32); be2 = np.asarray(inputs["be2"], np.float32)
    w1 = np.asarray(inputs["w1"], np.float32); b1 = np.asarray(inputs["b1"], np.float32)
    w2 = np.asarray(inputs["w2"], np.float32); b2 = np.asarray(inputs["b2"], np.float32)
    fc1_w = np.asarray(inputs["fc1_w"], np.float32)
    fc1_b = np.asarray(inputs["fc1_b"], np.float32)
    fc2_w = np.asarray(inputs["fc2_w"], np.float32)
    fc2_b = np.asarray(inputs["fc2_b"], np.float32)

    dft = _dft_consts()
    xf = x.reshape(B, TOKB, C)
    fc1s = g2[:, None] * fc1_w                                   # (768, 3072)
    fc1q = (fc1s * S1).astype(E4)
    fc1dr_m = np.ascontiguousarray(
        fc1q.reshape(3, 2, 128, HID).transpose(0, 2, 1, 3))      # [3,128,2,HID]
    fc2q = (fc2_w * S2).astype(E4)
    fc2dr_m = np.ascontiguousarray(
        fc2q.reshape(12, 2, 128, C).transpose(0, 2, 1, 3))       # [12,128,2,C]
    fc2bias_m = np.zeros((1, 2, C), np.float32)
    fc2bias_m[0, 0, :] = fc2_b * S2
    fc2bias_m = fc2bias_m.astype(E4)
    onesdr_m = np.ones((1, 2, TT), np.float32).astype(E4)
    gbias_v = (fc1_b + be2 @ fc1_w).astype(np.float32)           # (3072,)
    gbias_m = np.ascontiguousarray(gbias_v.reshape(NMO, 128).T)  # (128, 24)
    g1f_m = np.ascontiguousarray(g1.reshape(NCC, 128).T)
    be1f_m = np.ascontiguousarray(be1.reshape(NCC, 128).T)
    ones128 = np.ones((128, 1), BF)
    o1row_m = np.concatenate([np.ones((1, NM), np.float32),
                              np.zeros((1, NM), np.float32)], 0).astype(BF)

    in_maps = []
    for k in range(NCORES):
        ck = slice(k * BS, (k + 1) * BS)
        xw_k = np.ascontiguousarray(
            x[:, :, :, ck].transpose(2, 0, 1, 3)).astype(BF)     # [W,B,H,BS]
        xc_k = np.ascontiguousarray(
            np.concatenate([xf[0, k * TSB:(k + 1) * TSB],
                            xf[1, k * TSB:(k + 1) * TSB]], 0).T).astype(BF)
        g1k = g1[ck]
        w1r_k = w1[k, :, :, 0]; w1i_k = w1[k, :, :, 1]
        w1rp = (g1k[:, None] * w1r_k).astype(BF)
        w1ip = (g1k[:, None] * w1i_k).astype(BF)
        w1imp = (-(g1k[:, None] * w1i_k)).astype(BF)
        spike = (be1[ck] * SQN).astype(np.float64)
        b1sr_k = (w1r_k.T.astype(np.float64) @ spike).astype(np.float32)[:, None]
        b1si_k = (w1i_k.T.astype(np.float64) @ spike).astype(np.float32)[:, None]
        w2r_k = w2[k, :, :, 0]; w2i_k = w2[k, :, :, 1]
        b2pk_k = np.concatenate([b2[k, :, 0], b2[k, :, 1]])[None, :]
        w2p1_k = np.concatenate(
            [np.concatenate([w2r_k, w2i_k], 1), b2pk_k], 0).astype(BF)
        w2p2_k = np.concatenate(
            [np.concatenate([-w2i_k, w2r_k], 1),
             np.zeros((1, 192), np.float32)], 0).astype(BF)
        in_maps.append({
            "xw": xw_k, "xc": xc_k,
            **{n: dft[n] for n in ("fwp0", "fwp1", "f2a", "f2b", "iha", "ihb",
                                   "iwrt", "iwit")},
            "w1r": w1rp, "w1i": w1ip, "w1im": w1imp,
            "b1r": b1[k, :, 0:1].copy(), "b1i": b1[k, :, 1:2].copy(),
            "b1sr": b1sr_k, "b1si": b1si_k,
            "w2p1": w2p1_k, "w2p2": w2p2_k,
            "fc1dr": fc1dr_m, "fc2dr": fc2dr_m, "fc2bias": fc2bias_m,
            "onesdr": onesdr_m, "gbias": gbias_m,
            "g1f": g1f_m, "be1f": be1f_m,
            "ones128": ones128, "o1row": o1row_m,
        })
    return in_maps


def kernel(**inputs):
    g1 = np.asarray(inputs["g1"], np.float32)
    be1 = np.asarray(inputs["be1"], np.float32)
    simple = bool(np.all(g1 == 1.0) and np.all(be1 == 0.0))
    nc = _build_nc(simple)
    in_maps = _host_prep(inputs)
    res = run_bass_kernel_spmd(nc, in_maps, core_ids=list(range(NCORES)))
    full = np.empty((B, TOKB, C), np.float32)
    for j in range(NCORES):
        o = np.asarray(res.results[j]["out"], np.float32).T   # [4050, 768]
        full[0, j * TSB:(j + 1) * TSB] = o[:TSB]
        full[1, j * TSB:(j + 1) * TSB] = o[TSB:]
    return full.reshape(B, H, W, C)


# revision 12
# speedup vs baseline: 21255.8665x; 1.0160x over previous
"""AFNO transformer block on 8 Trainium2 NeuronCores (bf16 + fp8 MLP).

Distribution:
  Phase 1 (channel-block sharded): core k owns channels [96k, 96k+96).
    LN1 partial stats -> per-batch AllReduce -> LN1 applied in place
    (gpsimd, 24-channel blocks) -> spectral path: F1 (W-DFT, flip), F2
    (H-DFT, flip, packed), block complex MLP (layer-2 bias via an
    augmented 97th contraction row), inverse H-DFT, inverse W-DFT
    emitted channel-major (loop over h) so the AllToAll sends are 8
    contiguous [96, 2025] DMAs instead of ~9k 360-byte bursts.
  Phase-2 LN1 row stats for all 10 token tiles are precomputed in the
    kernel-start window (PE + gpsimd idle while AllReduce 0 is in
    flight) and parked in DRAM.
  Phase 2 (token sharded): core j owns tokens [2025j, 2025(j+1)).
    h = filt + LN1(x) + x in bf16; LN row vectors broadcast to 128
    partitions via a DRAM round-trip DMA (zero-stride read); fc1/fc2 in
    fp8 e4m3 DoubleRow mode (K=256/matmul), descale via activation
    scale; fc2 bias folded in as a K=1 DoubleRow matmul; residual adds
    on gpsimd. When g1==1 and be1==0 (detected at build time) the LN1
    apply folds into x*(rstd+1) - m*rstd: 3 vector ops per channel
    chunk. Tile (0,0) is prepped during b=1's spectral phase; later
    tiles pipeline 1-deep (prep of k+1 fully inside compute of k).
"""
import math
import numpy as np
import ml_dtypes

import concourse.bass as bass
import concourse.mybir as mybir
import concourse.tile as tile
from concourse import bacc
from concourse.bass_utils import run_bass_kernel_spmd

F32 = mybir.dt.float32
BF16 = mybir.dt.bfloat16
FP8 = mybir.dt.float8e4
AF = mybir.ActivationFunctionType
OP = mybir.AluOpType
AX = mybir.AxisListType
PM = mybir.MatmulPerfMode

NCORES = 8
B, H, W, C = 2, 90, 180, 768
BS = 96            # channels per core / AFNO block size
KW = 46            # kept W-frequency modes
HID = 3072
LAM = 0.01
EPS = 1e-5
TOKB = H * W       # 16200 tokens per batch
TSB = TOKB // NCORES   # 2025 tokens per (core, batch)
TSH = 2 * TSB      # 4050 tokens per core
NM = KW * H        # 4140 modes per block
SQN = math.sqrt(H * W)
NCC = 6            # 768/128
NMO = 24           # 3072/128
TT = 405           # phase-2 token tile width
NT = TSB // TT     # 5 tiles per batch
M1CH = 460         # MLP1 chunk (4140 = 9*460)
BF = ml_dtypes.bfloat16
E4 = ml_dtypes.float8_e4m3
S1 = 64.0          # fc1 fp8 weight scale
S2 = 128.0         # fc2 fp8 weight scale


def _dft_consts():
    wv = np.arange(W, dtype=np.float64)[:, None]
    wf = np.arange(KW, dtype=np.float64)[None, :]
    ang = 2.0 * np.pi * wv * wf / W
    fwr = np.cos(ang) / math.sqrt(W)
    fwi = -np.sin(ang) / math.sqrt(W)
    fwpack = np.concatenate([fwr, fwi], axis=1)          # (180, 92)
    hv = np.arange(H, dtype=np.float64)[:, None]
    hf = np.arange(H, dtype=np.float64)[None, :]
    angh = 2.0 * np.pi * hv * hf / H
    fhc = np.cos(angh) / math.sqrt(H)
    fhs = np.sin(angh) / math.sqrt(H)
    fhsm = -fhs
    alpha = np.ones(KW); alpha[1:] = 2.0
    iwr = alpha[None, :] * np.cos(ang) / math.sqrt(W)    # (180, 46)
    iwi = -alpha[None, :] * np.sin(ang) / math.sqrt(W)
    iwrt = np.ascontiguousarray(iwr.T)                   # (46, 180)
    iwit = np.ascontiguousarray(iwi.T)
    c = {}
    c["fwp0"] = fwpack[:90]
    c["fwp1"] = fwpack[90:]
    c["f2a"] = np.concatenate([fhc, fhsm], axis=1)       # (90, 180)
    c["f2b"] = np.concatenate([fhs, fhc], axis=1)
    c["iha"] = np.concatenate([fhc, fhs], axis=1)
    c["ihb"] = np.concatenate([fhsm, fhc], axis=1)
    c["iwrt"] = iwrt
    c["iwit"] = iwit
    return {k: np.ascontiguousarray(v).astype(BF) for k, v in c.items()}


def _recv_pieces(cc):
    c0, out, r0 = cc * 128, [], 0
    while r0 < 128:
        s = (c0 + r0) // BS
        ci = (c0 + r0) % BS
        n = min(BS - ci, 128 - r0)
        out.append((r0, s, ci, n))
        r0 += n
    return out


_CACHE = {}


def _build_nc(simple_ln1):
    key = ("nc", simple_ln1)
    if key in _CACHE:
        return _CACHE[key]
    nc = bacc.Bacc("TRN2", target_bir_lowering=False, debug=False,
                   num_devices=NCORES)

    def g(n, s, dt=BF16):
        return nc.dram_tensor(n, s, dt, kind="ExternalInput")

    xw = g("xw", [W, B, H, BS])
    xc = g("xc", [C, TSH])
    fwp0 = g("fwp0", [90, 92]); fwp1 = g("fwp1", [90, 92])
    f2a = g("f2a", [90, 180]); f2b = g("f2b", [90, 180])
    iha = g("iha", [90, 180]); ihb = g("ihb", [90, 180])
    iwrt = g("iwrt", [KW, W]); iwit = g("iwit", [KW, W])
    w1r = g("w1r", [BS, BS]); w1i = g("w1i", [BS, BS]); w1im = g("w1im", [BS, BS])
    b1r = g("b1r", [BS, 1], F32); b1i = g("b1i", [BS, 1], F32)
    b1sr = g("b1sr", [BS, 1], F32); b1si = g("b1si", [BS, 1], F32)
    w2p1 = g("w2p1", [BS + 1, 192]); w2p2 = g("w2p2", [BS + 1, 192])
    fc1dr = g("fc1dr", [3, 128, 2, HID], FP8)
    fc2dr = g("fc2dr", [12, 128, 2, C], FP8)
    fc2bias = g("fc2bias", [1, 2, C], FP8)
    onesdr = g("onesdr", [1, 2, TT], FP8)
    gbias = g("gbias", [128, NMO], F32)
    g1f = g("g1f", [128, NCC], F32); be1f = g("be1f", [128, NCC], F32)
    ones128 = g("ones128", [128, 1])
    o1row = g("o1row", [2, NM])

    out = nc.dram_tensor("out", [C, TSH], F32, kind="ExternalOutput")
    rg = [list(range(NCORES))]

    tiles = [(b_, t_) for b_ in range(B) for t_ in range(NT)]
    NTILES = len(tiles)

    from contextlib import ExitStack
    with tile.TileContext(nc) as tc:
        with ExitStack() as st0:
            cp = st0.enter_context(tc.tile_pool(name="const", bufs=1))
            dram = st0.enter_context(tc.tile_pool(name="dram", bufs=1,
                                                  space="DRAM"))
            dstg = st0.enter_context(tc.tile_pool(name="dstg", bufs=4,
                                                  space="DRAM"))
            # early-prep pools (tile k=0 + row-stat precompute)
            exct = st0.enter_context(tc.tile_pool(name="exct", bufs=2))
            ehtr = st0.enter_context(tc.tile_pool(name="ehtr", bufs=1))
            ehtb = st0.enter_context(tc.tile_pool(name="ehtb", bufs=1))
            escr = st0.enter_context(tc.tile_pool(name="escr", bufs=1))
            ebcp = st0.enter_context(tc.tile_pool(name="ebcp", bufs=1))
            ehtc = st0.enter_context(tc.tile_pool(name="ehtc", bufs=1))
            erw = st0.enter_context(tc.tile_pool(name="erw", bufs=2))
            pln = st0.enter_context(tc.tile_pool(name="pln", bufs=2,
                                                 space="PSUM"))

            def cl(t, shape, dt=BF16):
                nm = f"c_{t.name}"
                s = cp.tile(shape, dt, name=nm, tag=nm)
                nc.sync.dma_start(s[:], t[:])
                return s

            c_fwp0 = cl(fwp0, [90, 92]); c_fwp1 = cl(fwp1, [90, 92])
            c_f2a = cl(f2a, [90, 180]); c_f2b = cl(f2b, [90, 180])
            c_iha = cl(iha, [90, 180]); c_ihb = cl(ihb, [90, 180])
            c_iwrt = cl(iwrt, [KW, W]); c_iwit = cl(iwit, [KW, W])
            c_w1r = cl(w1r, [BS, BS]); c_w1i = cl(w1i, [BS, BS])
            c_w1im = cl(w1im, [BS, BS])
            c_b1r = cl(b1r, [BS, 1], F32); c_b1i = cl(b1i, [BS, 1], F32)
            c_b1sr = cl(b1sr, [BS, 1], F32); c_b1si = cl(b1si, [BS, 1], F32)
            c_w2p1 = cl(w2p1, [BS + 1, 192]); c_w2p2 = cl(w2p2, [BS + 1, 192])
            c_gbias = cl(gbias, [128, NMO], F32)
            c_g1f = cl(g1f, [128, NCC], F32); c_be1f = cl(be1f, [128, NCC], F32)
            c_ones128 = cl(ones128, [128, 1])
            c_fc2bias = cl(fc2bias, [1, 2, C], FP8)
            c_onesdr = cl(onesdr, [1, 2, TT], FP8)
            c_eps = cp.tile([128, 1], F32, name="c_eps")
            nc.vector.memset(c_eps[:], EPS)

            st_in = [dram.tile([2, TOKB], F32, name=f"st_in{b_}") for b_ in range(B)]
            st_out = [dram.tile([2, TOKB], F32, name=f"st_out{b_}") for b_ in range(B)]
            a2a_in = [dram.tile([NCORES, BS, TSB], BF16, name=f"a2a_in{b_}")
                      for b_ in range(B)]
            a2a_out = [dram.tile([NCORES, BS, TSB], BF16, name=f"a2a_out{b_}")
                       for b_ in range(B)]
            rows_dram = dram.tile([B, 2, TSB], F32, name="rows_dram")
            stg_e0 = dram.tile([2, TT], BF16, name="stg_e0")

            # ---------- shared helpers ----------
            def bcast_read(stg, pool, tag):
                bc = pool.tile([128, 2, TT], BF16, tag=tag, name=tag)
                nc.sync.dma_start(
                    bc[:], bass.AP(tensor=stg[:].tensor, offset=stg[:].offset,
                                   ap=[[0, 128], [TT, 2], [1, TT]]))
                return bc

            def stage_write(a, b_, stg):
                nc.sync.dma_start(stg[0:1, :], a[:])
                nc.sync.dma_start(stg[1:2, :], b_[:])

            def row_math(rm, rq, pool, plus1=False):
                """raw sums rm, sq-sums rq [1,TT] f32 -> bf16 row pair.

                Returns (rstd [+1 if plus1], mean*rstd)."""
                vt = pool.tile([1, TT], F32, tag="vt", name="vt")
                nc.vector.tensor_scalar(out=rm[:], in0=rm[:], scalar1=1.0 / C,
                                        scalar2=None, op0=OP.mult)
                nc.vector.tensor_scalar(out=rq[:], in0=rq[:], scalar1=1.0 / C,
                                        scalar2=None, op0=OP.mult)
                nc.vector.tensor_mul(vt[:], rm[:], rm[:])
                nc.vector.tensor_sub(rq[:], rq[:], vt[:])
                nc.scalar.activation(out=rq[:], in_=rq[:], func=AF.Sqrt,
                                     bias=c_eps[:1])
                nc.vector.reciprocal(rq[:], rq[:])
                r1w = pool.tile([1, TT], BF16, tag="r1w", name="r1w")
                mr1w = pool.tile([1, TT], BF16, tag="mr1w", name="mr1w")
                nc.vector.tensor_mul(rm[:], rm[:], rq[:])
                nc.vector.tensor_copy(mr1w[:], rm[:])
                if plus1:
                    nc.vector.tensor_scalar(out=r1w[:], in0=rq[:],
                                            scalar1=1.0, scalar2=None,
                                            op0=OP.add)
                else:
                    nc.vector.tensor_copy(r1w[:], rq[:])
                return r1w, mr1w

            def t1_chain(xall, htrall, rbc, p_scr, p_htb):
                """htb = filt + LN1(x) + x per channel chunk; LN2 sums."""
                t3 = p_scr.tile([128, NCC, TT], BF16, tag="big3", name="t3")
                htb = p_htb.tile([128, NCC, TT], BF16, tag="htb", name="htb")
                for cc in range(NCC):
                    ts = t3[:, cc, :]
                    nc.vector.tensor_mul(ts, xall[:, cc, :], rbc[:, 0, :])
                    nc.vector.tensor_sub(ts, ts, rbc[:, 1, :])
                    if not simple_ln1:
                        nc.vector.tensor_scalar(out=ts, in0=ts,
                                                scalar1=c_g1f[:, cc:cc + 1],
                                                scalar2=c_be1f[:, cc:cc + 1],
                                                op0=OP.mult, op1=OP.add)
                        nc.vector.tensor_add(ts, ts, xall[:, cc, :])
                    nc.vector.tensor_add(htb[:, cc, :], htrall[:, cc, :], ts)
                hq3 = p_scr.tile([128, NCC, TT], BF16, tag="big3", name="hq3")
                nc.scalar.activation(out=hq3[:], in_=htb[:], func=AF.Square)
                hs = p_scr.tile([128, TT], BF16, tag="hs", name="hs")
                hq = p_scr.tile([128, TT], BF16, tag="hq", name="hq")
                with nc.allow_low_precision(reason="bf16 stat partials"):
                    nc.vector.tensor_add(hs[:], htb[:, 0, :], htb[:, 1, :])
                    for cc in range(2, NCC):
                        nc.vector.tensor_add(hs[:], hs[:], htb[:, cc, :])
                    nc.vector.tensor_add(hq[:], hq3[:, 0, :], hq3[:, 1, :])
                    for cc in range(2, NCC):
                        nc.vector.tensor_add(hq[:], hq[:], hq3[:, cc, :])
                return htb, hs, hq

            def ln2_mm(hs, hq, p_ps):
                pss = p_ps.tile([1, TT], F32, tag="pst", name="pss")
                psq = p_ps.tile([1, TT], F32, tag="pst", name="psq")
                nc.tensor.matmul(pss[:], c_ones128[:], hs[:], start=True,
                                 stop=True)
                nc.tensor.matmul(psq[:], c_ones128[:], hq[:], start=True,
                                 stop=True)
                return pss, psq

            def ln2_fin(pss, psq, p_rw, stg, p_bc, tag):
                m2 = p_rw.tile([1, TT], F32, tag="m2", name="m2")
                r2 = p_rw.tile([1, TT], F32, tag="r2", name="r2")
                nc.vector.tensor_copy(m2[:], pss[:])
                nc.vector.tensor_copy(r2[:], psq[:])
                r2b, m2b = row_math(m2, r2, p_rw)
                stage_write(r2b, m2b, stg)
                return bcast_read(stg, p_bc, tag)

            def htc_make(htb, cbc, p_scr, p_htc):
                tmp3 = p_scr.tile([128, NCC, TT], BF16, tag="big3",
                                  name="tmp3")
                htcp = p_htc.tile([128, NCC, TT], FP8, tag="htcp",
                                  name="htcp")
                for cc in range(NCC):
                    nc.vector.tensor_mul(tmp3[:, cc, :], htb[:, cc, :],
                                         cbc[:, 0, :])
                    nc.vector.tensor_sub(htcp[:, cc, :], tmp3[:, cc, :],
                                         cbc[:, 1, :])
                return htcp

            def load_xall(pool, tg):
                xall = pool.tile([128, NCC, TT], BF16, tag="xall",
                                 name="xall")
                for cc in range(NCC):
                    nc.sync.dma_start(
                        xall[:, cc, :],
                        xc[cc * 128:(cc + 1) * 128, tg:tg + TT])
                return xall

            def load_htr(pool, tb, t0):
                htrall = pool.tile([128, NCC, TT], BF16, tag="htr",
                                   name="htr")
                for cc in range(NCC):
                    for (r0, sc, ci, n) in _recv_pieces(cc):
                        nc.sync.dma_start(
                            htrall[r0:r0 + n, cc, :],
                            a2a_out[tb][sc, ci:ci + n, t0:t0 + TT])
                return htrall

            # ================= phase 1 =================
            with ExitStack() as st1:
                sqp = st1.enter_context(tc.tile_pool(name="sqp", bufs=2))
                stp = st1.enter_context(tc.tile_pool(name="stats", bufs=1))
                zp = st1.enter_context(tc.tile_pool(name="zp", bufs=2))
                clp = st1.enter_context(tc.tile_pool(name="clp", bufs=2))
                bigp = st1.enter_context(tc.tile_pool(name="bigp", bufs=2))
                s2p = st1.enter_context(tc.tile_pool(name="s2p", bufs=1))
                pp = st1.enter_context(tc.tile_pool(name="psum1", bufs=6,
                                                    space="PSUM"))
                zhs = {}

                def stk(t, kind):
                    return bass.AP(tensor=t[:].tensor,
                                   offset=t[:].offset + kind * TOKB,
                                   ap=[[90, 90], [8100, 2], [1, 90]])

                def emit_loads_stats(b):
                    """Load z (bf16), partial LN1 stats, AllReduce trigger."""
                    eng = nc.vector
                    zh = []
                    for wc in range(2):
                        zt = zp.tile([90, H, BS], BF16, tag="zh",
                                     name=f"zh{b}{wc}")
                        eng_ld = nc.scalar if b == 0 else nc.sync
                        eng_ld.dma_start(
                            zt[:], xw[wc * 90:(wc + 1) * 90, b, :, :])
                        zh.append(zt)
                    zhs[b] = zh
                    s_sum = stp.tile([90, 2, H], F32, tag="ssum")
                    s_sq = stp.tile([90, 2, H], F32, tag="ssq")
                    s_t = stp.tile([90, H], F32, tag="st_t")
                    zhs[b, "sum"] = s_sum
                    zhs[b, "sq"] = s_sq
                    for wc in range(2):
                        zt = zh[wc]
                        eng.reduce_sum(s_sum[:, wc, :], zt[:], axis=AX.X)
                        for blk in range(4):
                            sqt = sqp.tile([90, H, 24], BF16, tag="sqt")
                            zsl = zt[:, :, blk * 24:(blk + 1) * 24]
                            if b == 0:
                                nc.scalar.activation(out=sqt[:], in_=zsl,
                                                     func=AF.Square)
                            else:
                                nc.gpsimd.tensor_mul(sqt[:], zsl, zsl)
                            if blk == 0:
                                eng.reduce_sum(s_sq[:, wc, :], sqt[:], axis=AX.X)
                            else:
                                eng.reduce_sum(s_t[:], sqt[:], axis=AX.X)
                                eng.tensor_add(s_sq[:, wc, :], s_sq[:, wc, :],
                                               s_t[:])
                    nc.sync.dma_start(stk(st_in[b], 0), s_sum[:])
                    nc.sync.dma_start(stk(st_in[b], 1), s_sq[:])
                    nc.gpsimd.collective_compute(
                        "AllReduce", OP.add, replica_groups=rg,
                        ins=[st_in[b][:].opt()], outs=[st_out[b][:].opt()])

                def emit_post_stats(b):
                    """st recv, m/r, LN1 apply in place (gpsimd)."""
                    s_sum, s_sq = zhs[b, "sum"], zhs[b, "sq"]
                    nc.sync.dma_start(s_sum[:], stk(st_out[b], 0))
                    nc.sync.dma_start(s_sq[:], stk(st_out[b], 1))
                    s_m = stp.tile([90, 2, H], F32, tag="sm")
                    s_r = stp.tile([90, 2, H], F32, tag="sr")
                    s_v = stp.tile([90, 2, H], F32, tag="sv")
                    nc.vector.tensor_scalar(out=s_m[:], in0=s_sum[:],
                                            scalar1=1.0 / C, scalar2=None,
                                            op0=OP.mult)
                    nc.vector.tensor_scalar(out=s_r[:], in0=s_sq[:],
                                            scalar1=1.0 / C, scalar2=None,
                                            op0=OP.mult)
                    nc.vector.tensor_mul(s_v[:], s_m[:], s_m[:])
                    nc.vector.tensor_sub(s_r[:], s_r[:], s_v[:])
                    nc.scalar.activation(out=s_r[:], in_=s_r[:],
                                         func=AF.Sqrt, bias=c_eps[:90])
                    nc.vector.reciprocal(s_r[:], s_r[:])
                    s_rb = stp.tile([90, 2, H], BF16, tag="srb")
                    s_mrb = stp.tile([90, 2, H], BF16, tag="smrb")
                    nc.vector.tensor_copy(s_rb[:], s_r[:])
                    nc.vector.tensor_mul(s_v[:], s_m[:], s_r[:])
                    nc.vector.tensor_copy(s_mrb[:], s_v[:])
                    zhs[b, "smrb"] = s_mrb

                    def bc(t, wc, n):
                        a = t[:, wc, :]
                        return bass.AP(tensor=a.tensor, offset=a.offset,
                                       ap=[list(a.ap[0]), [1, H], [0, n]])
                    for c0 in range(0, BS, 24):
                        for wc in range(2):
                            zv = zhs[b][wc][:, :, c0:c0 + 24]
                            nc.gpsimd.tensor_mul(zv, zv, bc(s_rb, wc, 24))

                def emit_f1(b):
                    zh = zhs[b]
                    s_mrb = zhs[b, "smrb"]
                    fmr = stp.tile([90, 92], F32, tag="fmr", name="fmr")
                    pfm = pp.tile([90, 92], F32, tag="pp", name="psfmr")
                    nc.tensor.matmul(pfm[:], s_mrb[:, 0, :], c_fwp0[:],
                                     start=True, stop=False)
                    nc.tensor.matmul(pfm[:], s_mrb[:, 1, :], c_fwp1[:],
                                     start=False, stop=True)
                    nc.vector.tensor_copy(fmr[:], pfm[:])
                    yb = bigp.tile([90, BS, 92], BF16, tag="big", name=f"yb{b}")
                    zhs[b, "yb"] = yb
                    for gi, c0 in enumerate(range(0, BS, 4)):
                        pf = pp.tile([90, 4 * 92], F32, tag="pp", name="psf1")
                        for ci in range(4):
                            c = c0 + ci
                            nc.tensor.matmul(pf[:, ci * 92:(ci + 1) * 92],
                                             zh[0][:, :, c], c_fwp0[:],
                                             start=True, stop=False)
                            nc.tensor.matmul(pf[:, ci * 92:(ci + 1) * 92],
                                             zh[1][:, :, c], c_fwp1[:],
                                             start=False, stop=True)
                        dst = yb[:, c0:c0 + 4, :]
                        src = pf[:].rearrange("p (a b) -> p a b", a=4)
                        fa = fmr[:]
                        fbc = bass.AP(tensor=fa.tensor, offset=fa.offset,
                                      ap=[list(fa.ap[0]), [0, 4], [1, 92]])
                        nc.vector.tensor_tensor(out=dst, in0=src, in1=fbc,
                                                op=OP.subtract)

                def emit_f2(b):
                    yb = zhs[b, "yb"]
                    zb = bigp.tile([BS, 2, KW, H], BF16, tag="big",
                                   name=f"zb{b}")
                    zhs[b, "zb"] = zb
                    for wf in range(KW):
                        pz = pp.tile([BS, 180], F32, tag="pp", name="psf2")
                        nc.tensor.matmul(pz[:], yb[:, :, wf], c_f2a[:],
                                         start=True, stop=False)
                        nc.tensor.matmul(pz[:], yb[:, :, 46 + wf], c_f2b[:],
                                         start=False, stop=True)
                        if wf % 2 == 0:
                            nc.scalar.activation(
                                out=zb[:, :, wf, :],
                                in_=pz[:].rearrange("p (a b) -> p a b", a=2),
                                func=AF.Copy)
                        else:
                            nc.vector.tensor_copy(
                                zb[:, :, wf, :],
                                pz[:].rearrange("p (a b) -> p a b", a=2))

                def emit_mlp(b):
                    zb = zhs[b, "zb"]
                    o1 = bigp.tile([BS + 1, 2, NM], BF16, tag="big",
                                   name=f"o1{b}")
                    nc.sync.dma_start(o1[96:97, :, :], o1row[:])
                    zr_f = zb[:, 0].rearrange("p a b -> p (a b)")
                    zi_f = zb[:, 1].rearrange("p a b -> p (a b)")
                    for ch in range(9):
                        n0 = ch * M1CH
                        zr_s = zr_f[:, n0:n0 + M1CH]
                        zi_s = zi_f[:, n0:n0 + M1CH]
                        por = pp.tile([BS, M1CH], F32, tag="pp", name="pso1r")
                        nc.tensor.matmul(por[:], c_w1r[:], zr_s,
                                         start=True, stop=False)
                        nc.tensor.matmul(por[:], c_w1im[:], zi_s,
                                         start=False, stop=True)
                        poi = pp.tile([BS, M1CH], F32, tag="pp", name="pso1i")
                        nc.tensor.matmul(poi[:], c_w1i[:], zr_s,
                                         start=True, stop=False)
                        nc.tensor.matmul(poi[:], c_w1r[:], zi_s,
                                         start=False, stop=True)
                        if ch == 0:
                            # be1 spike contribution on mode (0,0) only
                            nc.vector.tensor_scalar(out=por[:, 0:1],
                                                    in0=por[:, 0:1],
                                                    scalar1=c_b1sr[:],
                                                    scalar2=None, op0=OP.add)
                            nc.vector.tensor_scalar(out=poi[:, 0:1],
                                                    in0=poi[:, 0:1],
                                                    scalar1=c_b1si[:],
                                                    scalar2=None, op0=OP.add)
                        nc.scalar.activation(out=o1[0:BS, 0, n0:n0 + M1CH],
                                             in_=por[:], func=AF.Relu,
                                             bias=c_b1r[:])
                        nc.scalar.activation(out=o1[0:BS, 1, n0:n0 + M1CH],
                                             in_=poi[:], func=AF.Relu,
                                             bias=c_b1i[:])

                    o2 = bigp.tile([H, 2, KW, BS], BF16, tag="big",
                                   name=f"o2{b}")
                    zhs[b, "o2"] = o2
                    for wf in range(KW):
                        lr = o1[:, 0, wf * H:(wf + 1) * H]
                        li = o1[:, 1, wf * H:(wf + 1) * H]
                        pm = pp.tile([H, 192], F32, tag="pp", name="pso2")
                        nc.tensor.matmul(pm[:], lr, c_w2p1[:],
                                         start=True, stop=False)
                        nc.tensor.matmul(pm[:], li, c_w2p2[:],
                                         start=False, stop=True)
                        clip = clp.tile([H, 192], F32, tag="clip")
                        nc.vector.tensor_scalar(out=clip[:], in0=pm[:],
                                                scalar1=-LAM, scalar2=LAM,
                                                op0=OP.max, op1=OP.min)
                        nc.vector.tensor_tensor(
                            out=o2[:, :, wf, :],
                            in0=pm[:].rearrange("p (a b) -> p a b", a=2),
                            in1=clip[:].rearrange("p (a b) -> p a b", a=2),
                            op=OP.subtract)

                def emit_ih(b):
                    o2 = zhs[b, "o2"]
                    u2 = bigp.tile([KW, 2, BS, H], BF16, tag="big",
                                   name=f"u2{b}")
                    zhs[b, "u2"] = u2
                    for c in range(BS):
                        lr = o2[:, 0, :, c]
                        li = o2[:, 1, :, c]
                        pu = pp.tile([KW, 180], F32, tag="pp", name="psu")
                        nc.tensor.matmul(pu[:], lr, c_iha[:],
                                         start=True, stop=False)
                        nc.tensor.matmul(pu[:], li, c_ihb[:],
                                         start=False, stop=True)
                        dst = u2[:, :, c, :]
                        src = pu[:].rearrange("p (a b) -> p a b", a=2)
                        if c % 2 == 0:
                            nc.scalar.activation(out=dst, in_=src, func=AF.Copy)
                        else:
                            nc.vector.tensor_copy(dst, src)

                def emit_iw(b):
                    """inverse W-DFT, channel-major; contiguous a2a sends."""
                    u2 = zhs[b, "u2"]
                    s2c = s2p.tile([BS, H, W], BF16, tag="s2", name=f"s2{b}")
                    for h in range(H):
                        pf = pp.tile([BS, W], F32, tag="pp", name="psw")
                        nc.tensor.matmul(pf[:], u2[:, 0, :, h], c_iwrt[:],
                                         start=True, stop=False)
                        nc.tensor.matmul(pf[:], u2[:, 1, :, h], c_iwit[:],
                                         start=False, stop=True)
                        if h % 2 == 0:
                            nc.scalar.activation(out=s2c[:, h, :], in_=pf[:],
                                                 func=AF.Copy)
                        else:
                            nc.vector.tensor_copy(s2c[:, h, :], pf[:])
                    flat = s2c[:].rearrange("p a b -> p (a b)")
                    for j in range(NCORES):
                        dst = bass.AP(
                            tensor=a2a_in[b][:].tensor,
                            offset=a2a_in[b][:].offset + j * BS * TSB,
                            ap=[[TSB, BS], [1, TSB]])
                        nc.scalar.dma_start(dst, flat[:, j * TSB:(j + 1) * TSB])

                def p2rows_all():
                    """LN1 row stats for every phase-2 tile, parked in DRAM.

                    Runs in the kernel-start window while AllReduce 0 is
                    in flight (PE/gpsimd otherwise idle)."""
                    for kk, (tb, tt) in enumerate(tiles):
                        tg = tb * TSB + tt * TT
                        xall = load_xall(exct, tg)
                        xq3 = escr.tile([128, NCC, TT], BF16, tag="big3",
                                        name="xq3")
                        nc.gpsimd.tensor_mul(xq3[:], xall[:], xall[:])
                        ps_m = pln.tile([1, TT], F32, tag="pst", name="psm")
                        ps_q = pln.tile([1, TT], F32, tag="pst", name="psq")
                        for cc in range(NCC):
                            nc.tensor.matmul(ps_m[:], c_ones128[:],
                                             xall[:, cc, :],
                                             start=(cc == 0),
                                             stop=(cc == NCC - 1))
                        for cc in range(NCC):
                            nc.tensor.matmul(ps_q[:], c_ones128[:],
                                             xq3[:, cc, :],
                                             start=(cc == 0),
                                             stop=(cc == NCC - 1))
                        rm = erw.tile([1, TT], F32, tag="rmu", name="rmu")
                        rq = erw.tile([1, TT], F32, tag="rqu", name="rqu")
                        nc.vector.tensor_copy(rm[:], ps_m[:])
                        nc.vector.tensor_copy(rq[:], ps_q[:])
                        if kk == 0:
                            r1w, mr1w = row_math(rm, rq, erw,
                                                 plus1=simple_ln1)
                            stage_write(r1w, mr1w, stg_e0)
                        else:
                            t0 = tt * TT
                            nc.scalar.dma_start(
                                rows_dram[tb, 0, t0:t0 + TT], rm[:])
                            nc.scalar.dma_start(
                                rows_dram[tb, 1, t0:t0 + TT], rq[:])

                # ---- phase-1 emission schedule ----
                emit_loads_stats(0)
                p2rows_all()
                emit_post_stats(0)
                emit_f1(0)
                emit_f2(0)
                emit_loads_stats(1)     # AR1 trigger
                emit_mlp(0)
                emit_ih(0)
                emit_post_stats(1)      # apply b=1 on gpsimd, pre-trigger
                emit_iw(0)
                nc.gpsimd.collective_compute(
                    "AllToAll", OP.bypass, replica_groups=rg,
                    ins=[a2a_in[0][:].opt()], outs=[a2a_out[0][:].opt()])
                emit_f1(1)
                emit_f2(1)
                exall0 = load_xall(exct, 0)
                ehtr0 = load_htr(ehtr, 0, 0)
                ebc0 = bcast_read(stg_e0, ebcp, "ebc")
                emit_mlp(1)
                ehtb0, ehs0, ehq0 = t1_chain(exall0, ehtr0, ebc0, escr, ehtb)
                emit_ih(1)
                pss0, psq0 = ln2_mm(ehs0, ehq0, pln)
                stgl0 = dstg.tile([2, TT], BF16, tag="stgl", name="stgl0")
                cbc0 = ln2_fin(pss0, psq0, erw, stgl0, ebcp, "ecbc")
                htcp0 = htc_make(ehtb0, cbc0, escr, ehtc)
                emit_iw(1)
                nc.gpsimd.collective_compute(
                    "AllToAll", OP.bypass, replica_groups=rg,
                    ins=[a2a_in[1][:].opt()], outs=[a2a_out[1][:].opt()])

            # ================= phase 2 =================
            with ExitStack() as st2:
                fc1p = st2.enter_context(tc.tile_pool(name="fc1p", bufs=1))
                fc2p = st2.enter_context(tc.tile_pool(name="fc2p", bufs=1))
                xtp = st2.enter_context(tc.tile_pool(name="xtp", bufs=2))
                hrp = st2.enter_context(tc.tile_pool(name="hrp", bufs=2))
                hbp = st2.enter_context(tc.tile_pool(name="hbp", bufs=2))
                hip = st2.enter_context(tc.tile_pool(name="hip", bufs=1))
                scp = st2.enter_context(tc.tile_pool(name="scp", bufs=1))
                hcp = st2.enter_context(tc.tile_pool(name="hcp", bufs=2))
                bcp = st2.enter_context(tc.tile_pool(name="bcp", bufs=2))
                rwp = st2.enter_context(tc.tile_pool(name="rwp", bufs=1))
                outp = st2.enter_context(tc.tile_pool(name="outp", bufs=2))
                pst = st2.enter_context(tc.tile_pool(name="pst", bufs=2,
                                                     space="PSUM"))
                pc = st2.enter_context(tc.tile_pool(name="pc", bufs=4,
                                                    space="PSUM"))

                c_fc1 = [fc1p.tile([128, 2, HID], FP8, tag=f"fc1_{i}",
                                   name=f"cfc1_{i}") for i in range(3)]
                for i in range(3):
                    nc.sync.dma_start(c_fc1[i][:], fc1dr[i])
                c_fc2 = [fc2p.tile([128, 2, C], FP8, tag=f"fc2_{i}",
                                   name=f"cfc2_{i}") for i in range(12)]
                for i in range(12):
                    nc.sync.dma_start(c_fc2[i][:], fc2dr[i])

                def pA_load(k):
                    tb, tt = tiles[k]
                    return (load_xall(xtp, tb * TSB + tt * TT),
                            load_htr(hrp, tb, tt * TT))

                def pA_rest(k, xall, htrall):
                    tb, tt = tiles[k]
                    rm = rwp.tile([1, TT], F32, tag="rm", name="rm")
                    rq = rwp.tile([1, TT], F32, tag="rq", name="rq")
                    t0 = tt * TT
                    nc.sync.dma_start(rm[:], rows_dram[tb, 0, t0:t0 + TT])
                    nc.sync.dma_start(rq[:], rows_dram[tb, 1, t0:t0 + TT])
                    r1w, mr1w = row_math(rm, rq, rwp, plus1=simple_ln1)
                    stg = dstg.tile([2, TT], BF16, tag="stgr", name="stgr")
                    stage_write(r1w, mr1w, stg)
                    rbc = bcast_read(stg, bcp, "rbc")
                    htb, hs, hq = t1_chain(xall, htrall, rbc, scp, hbp)
                    pss, psq2 = ln2_mm(hs, hq, pst)
                    return htb, pss, psq2

                def pB(k, htb, pss, psq):
                    stg = dstg.tile([2, TT], BF16, tag="stgl", name="stgl")
                    cbc = ln2_fin(pss, psq, rwp, stg, bcp, "cbc")
                    htcp = htc_make(htb, cbc, scp, hcp)
                    return htb, htcp

                def pC1(k, htb, htcp):
                    hid = hip.tile([128, NMO, TT], FP8, tag="hid", name="hid")
                    for mo in range(NMO):
                        php = pc.tile([128, TT], F32, tag="pc", name="php")
                        for ccp in range(3):
                            nc.tensor.matmul(
                                php[:],
                                c_fc1[ccp][:, :, mo * 128:(mo + 1) * 128],
                                htcp[:, 2 * ccp:2 * ccp + 2, :],
                                start=(ccp == 0), stop=(ccp == 2),
                                perf_mode=PM.DoubleRow)
                        nc.scalar.activation(out=hid[:, mo, :], in_=php[:],
                                             func=AF.Gelu,
                                             bias=c_gbias[:, mo:mo + 1],
                                             scale=1.0 / S1)
                    return hid

                def pC2(k, htb, htcp, hid):
                    tb, tt = tiles[k]
                    tg = tb * TSB + tt * TT
                    for co in range(NCC):
                        pop = pc.tile([128, TT], F32, tag="pc", name="pop")
                        for ho in range(12):
                            nc.tensor.matmul(
                                pop[:],
                                c_fc2[ho][:, :, co * 128:(co + 1) * 128],
                                hid[:, 2 * ho:2 * ho + 2, :],
                                start=(ho == 0), stop=False,
                                perf_mode=PM.DoubleRow)
                        nc.tensor.matmul(
                            pop[:], c_fc2bias[:, :, co * 128:(co + 1) * 128],
                            c_onesdr[:], start=False, stop=True,
                            perf_mode=PM.DoubleRow)
                        oft = outp.tile([128, TT], BF16, tag="oft", name="oft")
                        nc.scalar.activation(out=oft[:], in_=pop[:],
                                             func=AF.Copy, scale=1.0 / S2)
                        of = outp.tile([128, TT], F32, tag="of", name="of")
                        nc.gpsimd.tensor_add(of[:], oft[:], htb[:, co, :])
                        nc.sync.dma_start(
                            out[co * 128:(co + 1) * 128, tg:tg + TT], of[:])

                stB = {0: (ehtb0, htcp0)}
                Ld = {1: pA_load(1)}
                for k in range(NTILES):
                    if k + 2 < NTILES:
                        Ld[k + 2] = pA_load(k + 2)
                    hid = pC1(k, *stB[k])
                    stA = pA_rest(k + 1, *Ld[k + 1]) if k + 1 < NTILES else None
                    pC2(k, *stB[k], hid)
                    if stA is not None:
                        stB[k + 1] = pB(k + 1, *stA)

    nc.compile()
    _CACHE[key] = nc
    return nc


def _host_prep(inputs):
    x = np.ascontiguousarray(np.asarray(inputs["x"], dtype=np.float32))
    g1 = np.asarray(inputs["g1"], np.float32); be1 = np.asarray(inputs["be1"], np.float32)
    g2 = np.asarray(inputs["g2"], np.float32); be2 = np.asarray(inputs["be2"], np.float32)
    w1 = np.asarray(inputs["w1"], np.float32); b1 = np.asarray(inputs["b1"], np.float32)
    w2 = np.asarray(inputs["w2"], np.float32); b2 = np.asarray(inputs["b2"], np.float32)
    fc1_w = np.asarray(inputs["fc1_w"], np.float32)
    fc1_b = np.asarray(inputs["fc1_b"], np.float32)
    fc2_w = np.asarray(inputs["fc2_w"], np.float32)
    fc2_b = np.asarray(inputs["fc2_b"], np.float32)

    dft = _dft_consts()
    xf = x.reshape(B, TOKB, C)
    fc1s = g2[:, None] * fc1_w                                   # (768, 3072)
    fc1q = (fc1s * S1).astype(E4)
    fc1dr_m = np.ascontiguousarray(
        fc1q.reshape(3, 2, 128, HID).transpose(0, 2, 1, 3))      # [3,128,2,HID]
    fc2q = (fc2_w * S2).astype(E4)
    fc2dr_m = np.ascontiguousarray(
        fc2q.reshape(12, 2, 128, C).transpose(0, 2, 1, 3))       # [12,128,2,C]
    fc2bias_m = np.zeros((1, 2, C), np.float32)
    fc2bias_m[0, 0, :] = fc2_b * S2
    fc2bias_m = fc2bias_m.astype(E4)
    onesdr_m = np.ones((1, 2, TT), np.float32).astype(E4)
    gbias_v = (fc1_b + be2 @ fc1_w).astype(np.float32)           # (3072,)
    gbias_m = np.ascontiguousarray(gbias_v.reshape(NMO, 128).T)  # (128, 24)
    g1f_m = np.ascontiguousarray(g1.reshape(NCC, 128).T)
    be1f_m = np.ascontiguousarray(be1.reshape(NCC, 128).T)
    ones128 = np.ones((128, 1), BF)
    o1row_m = np.concatenate([np.ones((1, NM), np.float32),
                              np.zeros((1, NM), np.float32)], 0).astype(BF)

    in_maps = []
    for k in range(NCORES):
        ck = slice(k * BS, (k + 1) * BS)
        xw_k = np.ascontiguousarray(
            x[:, :, :, ck].transpose(2, 0, 1, 3)).astype(BF)     # [W,B,H,BS]
        xc_k = np.ascontiguousarray(
            np.concatenate([xf[0, k * TSB:(k + 1) * TSB],
                            xf[1, k * TSB:(k + 1) * TSB]], 0).T).astype(BF)
        g1k = g1[ck]
        w1r_k = w1[k, :, :, 0]; w1i_k = w1[k, :, :, 1]
        w1rp = (g1k[:, None] * w1r_k).astype(BF)
        w1ip = (g1k[:, None] * w1i_k).astype(BF)
        w1imp = (-(g1k[:, None] * w1i_k)).astype(BF)
        spike = (be1[ck] * SQN).astype(np.float64)
        b1sr_k = (w1r_k.T.astype(np.float64) @ spike).astype(np.float32)[:, None]
        b1si_k = (w1i_k.T.astype(np.float64) @ spike).astype(np.float32)[:, None]
        w2r_k = w2[k, :, :, 0]; w2i_k = w2[k, :, :, 1]
        b2pk_k = np.concatenate([b2[k, :, 0], b2[k, :, 1]])[None, :]
        w2p1_k = np.concatenate(
            [np.concatenate([w2r_k, w2i_k], 1), b2pk_k], 0).astype(BF)
        w2p2_k = np.concatenate(
            [np.concatenate([-w2i_k, w2r_k], 1),
             np.zeros((1, 192), np.float32)], 0).astype(BF)
        in_maps.append({
            "xw": xw_k, "xc": xc_k,
            **{n: dft[n] for n in ("fwp0", "fwp1", "f2a", "f2b", "iha", "ihb",
                                   "iwrt", "iwit")},
            "w1r": w1rp, "w1i": w1ip, "w1im": w1imp,
            "b1r": b1[k, :, 0:1].copy(), "b1i": b1[k, :, 1:2].copy(),
            "b1sr": b1sr_k, "b1si": b1si_k,
            "w2p1": w2p1_k, "w2p2": w2p2_k,
            "fc1dr": fc1dr_m, "fc2dr": fc2dr_m, "fc2bias": fc2bias_m,
            "onesdr": onesdr_m, "gbias": gbias_m,
            "g1f": g1f_m, "be1f": be1f_m,
            "ones128": ones128, "o1row": o1row_m,
        })
    return in_maps


def kernel(**inputs):
    g1 = np.asarray(inputs["g1"], np.float32)
    be1 = np.asarray(inputs["be1"], np.float32)
    simple = bool(np.all(g1 == 1.0) and np.all(be1 == 0.0))
    nc = _build_nc(simple)
    in_maps = _host_prep(inputs)
    res = run_bass_kernel_spmd(nc, in_maps, core_ids=list(range(NCORES)))
    full = np.empty((B, TOKB, C), np.float32)
    for j in range(NCORES):
        o = np.asarray(res.results[j]["out"], np.float32).T   # [4050, 768]
        full[0, j * TSB:(j + 1) * TSB] = o[:TSB]
        full[1, j * TSB:(j + 1) * TSB] = o[TSB:]
    return full.reshape(B, H, W, C)
